# revision 47
# baseline (speedup 1.0000x reference)
"""Trainium2 Bass kernel for nn_DTransformer (sparse attention w/ distance decay).

Sharding: data-parallel over batch (bs=8 -> 8 cores, weights replicated).

v2 design notes:
- All PE matmuls in bf16 (fp32 is 4 cyc/row on the PE; bf16 is 1).
- Causal mask folded into the zs write (STT on the diag block) - no PE mask
  matmuls; no rank-1 bias matmuls (residuals pre-biased on host, alpha of the
  final mixture computed on host, blv added on vector).
- A@V weight transposes done by the DMA xbar (dma_start_transpose), one
  descriptor per (head, q-tile), not by the PE.
- Distance-decay eff computed at full resolution only in a 256-wide diagonal
  band; beyond that on a stride-16 coarse grid (validated: 3e-5 output err).
  Suffix masses: segmented reversed scan over a compacted band (poison-mask
  trick resets at head boundaries) chained with a tiny scan over coarse block
  sums built by pairwise adds on the (otherwise idle) GPSIMD engine.
- sqrt via exp(0.5*ln(u)): ln+exp live in one ACT table set, so the scalar
  engine never swaps tables inside the hot loop (sqrt would force 2 swaps per
  q-tile). LayerNorm's 1/std likewise uses exp(-0.5*ln(var+eps)).
- zs / t / block-sum elementwise passes run on GPSIMD (Pool) to unload the
  vector engine.
"""

import os
import sys
import contextlib

for _p in ("/opt/trn_rl_repo", "/root/.axon_site/_ro/trn_rl_repo"):
    if os.path.isdir(_p) and _p not in sys.path:
        sys.path.insert(0, _p)

import numpy as np
import ml_dtypes

import concourse.bass as bass
import concourse.mybir as mybir
import concourse.tile as tile
from concourse import bacc

F32 = mybir.dt.float32
F16 = mybir.dt.float16
BF16 = mybir.dt.bfloat16
AF = mybir.ActivationFunctionType
OP = mybir.AluOpType

D = 256
H = 8
HG = 4            # heads per group
NG = H // HG
DK = 32
SEQ = 1024
BS = 8
NQT = SEQ // 128
ISQ = float(1.0 / np.sqrt(np.float32(DK)))
MASKF = -53000.0   # fits f16; exp() underflows to exactly 0
EPS = 1e-5
BAND = 128         # full-res band width (= diag block)
CF = 16            # coarse cell width

bf16 = ml_dtypes.bfloat16
f16 = np.float16
KEEP0 = frozenset({0})


def _opt(ap):
    return ap.opt(keep_dims=KEEP0)


def _rev(ap):
    """Reverse the innermost free dim of an AP (squeeze count-1 dims)."""
    pairs = [list(x) for x in ap.ap]
    keep = [pairs[0]] + [x for x in pairs[1:] if x[1] != 1]
    assert len(keep) == 2, f"need 2D-able ap, got {ap.ap}"
    (ps, pc), (fs, fc) = keep
    return bass.AP(tensor=ap.tensor, offset=ap.offset + fs * (fc - 1),
                   ap=[[ps, pc], [-fs, fc]])


def _ap(t, offset, dims):
    """Build an AP on tile t's tensor with explicit [stride, count] dims."""
    base = t[:]
    return bass.AP(tensor=base.tensor, offset=base.offset + offset,
                   ap=[list(base.ap[0])] + [list(d) for d in dims])


def _bc(ap, n):
    pairs = [list(x) for x in ap.ap]
    return bass.AP(tensor=ap.tensor, offset=ap.offset, ap=pairs + [[0, n]])


# ---------------------------------------------------------------- host prep

def host_prep(inputs):
    g = {k: np.asarray(v) for k, v in inputs.items()}

    def f32(x):
        return np.ascontiguousarray(np.asarray(x, dtype=np.float32))

    def as_bf(x):
        return np.ascontiguousarray(np.asarray(x, np.float32).astype(bf16))

    drv = {}
    for i, names in ((1, ("q", "v", "o")), (2, ("q", "v", "o")),
                     (3, ("k", "v", "o"))):
        for n in names:
            drv[f"WT_{n}{i}"] = as_bf(g[f"W{n}{i}"].T)      # [din, dout]
    for nm in ("bq1", "bq2", "bk3"):
        drv[nm + "_c"] = f32(np.asarray(g[nm], np.float32).reshape(2, 128).T)
    for nm in ("bv1", "bv2", "bv3"):
        drv[nm + "_r"] = f32(g[nm]).reshape(1, D)
    for i in (1, 2, 3):
        drv[f"lng{i}_r"] = f32(g[f"lng{i}"]).reshape(1, D)
        drv[f"lnb{i}_r"] = f32(g[f"lnb{i}"]).reshape(1, D)
        gam = -np.logaddexp(0.0, f32(g[f"g{i}"]).reshape(H))
        drv[f"gam2_{i}"] = f32((gam * gam).reshape(1, H))
    know = f32(g["know"]).reshape(D)
    q3 = know @ f32(g["Wq3"]).T + f32(g["bq3"])
    q3blk = np.zeros((D, H), np.float32)
    for h in range(H):
        q3blk[h * DK:(h + 1) * DK, h] = q3[h * DK:(h + 1) * DK]
    drv["q3blk"] = as_bf(q3blk)
    drv["knowr_r"] = f32(know + f32(g["bo3"])).reshape(1, D)  # pre-biased res
    # block-diagonal Wlv for the final per-head projection as ONE matmul
    WlvT = f32(g["Wlv"]).T                                    # [DK, D]
    Wblk = np.zeros((D, H * D), np.float32)
    for h in range(H):
        Wblk[h * DK:(h + 1) * DK, h * D:(h + 1) * D] = WlvT
    drv["Wblk"] = as_bf(Wblk)                                 # [256, 2048]
    drv["blv8"] = as_bf(np.tile(f32(g["blv"]), H)).reshape(1, H * D)

    p = np.arange(128, dtype=np.float32)[:, None]
    j1 = np.arange(128, dtype=np.float32)[None, :]
    posb1 = np.maximum(p - j1, 0.0)                  # band == diag block
    drv["POSB41"] = np.ascontiguousarray(
        np.tile(posb1, (1, HG)).astype(bf16))        # [128, 4*128]
    # descending coarse pos table: col jj -> 16*(64-jj) + p - 7.5; the
    # per-qt slice [64-8qt : 64-8qt+nm] walks cells m=0..nm-1 ascending
    jj = np.arange(72, dtype=np.float32)[None, :]
    drv["POSCR"] = np.ascontiguousarray(
        (CF * (64.0 - jj) + p - (CF - 1) / 2.0).astype(bf16))  # [128, 72]

    segb1 = np.ones((128, HG * 128), np.float32)
    segb1[:, 127::128] = 0.0
    drv["SEGB1"] = np.ascontiguousarray(segb1.astype(bf16))
    for qt in range(1, NQT):
        nm = qt * 128 // CF
        sc = np.ones((128, HG * (nm + 1)), np.float32)
        sc[:, nm::nm + 1] = 0.0
        drv[f"SEGC{qt}"] = np.ascontiguousarray(sc.astype(bf16))
    drv["ONES1"] = as_bf(np.ones((1, 128)))

    jj = np.arange(128)[None, :]
    drv["M0s"] = np.ascontiguousarray(
        np.where(jj <= np.arange(128)[:, None], 0.0, MASKF).astype(f16))
    drv["M3"] = np.ascontiguousarray(
        np.where(jj < np.arange(128)[:, None], 0.0, -6e4).astype(f16))
    drv["IDF"] = f32(np.eye(128))

    # host-side alpha for the final mixture (tiny: [bs, s, h])
    kk = know.reshape(H, DK) @ f32(g["Wlk"]).T + f32(g["blk"])
    kk = 1.0 / (1.0 + np.exp(-kk))
    q = np.asarray(g["q_emb"], np.float32)
    beta = np.einsum("hd,bsd->bsh", kk, q)
    beta -= beta.max(-1, keepdims=True)
    ee = np.exp(beta)
    drv["_alpha"] = (ee / ee.sum(-1, keepdims=True)).astype(np.float32)
    return drv


def per_batch_maps(inputs, drv):
    q = np.asarray(inputs["q_emb"], np.float32)
    s = np.asarray(inputs["s_emb"], np.float32)
    bo1 = np.asarray(inputs["bo1"], np.float32)
    bo2 = np.asarray(inputs["bo2"], np.float32)
    alpha = drv["_alpha"]
    base = {k: v for k, v in drv.items() if not k.startswith("_")}
    maps = []
    for b in range(BS):
        m = dict(base)
        m["xbf_q"] = np.ascontiguousarray(q[b].astype(bf16))
        m["xbf_s"] = np.ascontiguousarray(s[b].astype(bf16))
        m["xr_q"] = np.ascontiguousarray(q[b] + bo1)
        m["xr_s"] = np.ascontiguousarray(s[b] + bo2)
        m["al_nat"] = np.ascontiguousarray(alpha[b])
        maps.append(m)
    return maps


# ---------------------------------------------------------------- builder

class KB:
    def __init__(self, nc, tc, ctx):
        self.nc, self.tc, self.ctx = nc, tc, ctx

    def pst(self, shape):
        return self.pps.tile(shape, F32, tag="ps", name="ps")

    def load_consts(self, dd):
        nc = self.nc
        pool = self.ctx.enter_context(self.tc.tile_pool(name="consts", bufs=1))
        sb = {}
        for i, names in ((1, ("q", "v", "o")), (2, ("q", "v", "o")),
                         (3, ("k", "v", "o"))):
            for n in names:
                t = pool.tile([128, 2, D], BF16, tag=f"WT_{n}{i}")
                nc.sync.dma_start(
                    out=t[:],
                    in_=dd[f"WT_{n}{i}"][:].rearrange("(a p) d -> p a d", p=128))
                sb[f"WT_{n}{i}"] = t
        t = pool.tile([128, 2, H], BF16, tag="q3blk")
        nc.sync.dma_start(
            out=t[:], in_=dd["q3blk"][:].rearrange("(a p) h -> p a h", p=128))
        sb["q3blk"] = t
        for nm in list(dd.keys()):
            if nm.startswith(("POSB", "POSC", "SEGB", "SEGC", "M0s", "M3",
                              "IDF")) or nm.endswith("_c"):
                src = dd[nm]
                t = pool.tile(list(src.shape), src.dtype, tag=nm)
                nc.sync.dma_start(out=t[:], in_=src[:])
                sb[nm] = t
        for nm in ("bv1_r", "bv2_r", "bv3_r", "lng1_r", "lng2_r", "lng3_r",
                   "lnb1_r", "lnb2_r", "lnb3_r", "knowr_r", "gam2_1",
                   "gam2_2", "gam2_3"):
            src = dd[nm]
            n = src.shape[1]
            t = pool.tile([128, n], F32, tag=nm)
            nc.sync.dma_start(
                out=t[:],
                in_=bass.AP(tensor=src, offset=0, ap=[[0, 128], [1, n]]))
            sb[nm] = t
        t = pool.tile([1, 128], BF16, tag="ONES1")
        nc.sync.dma_start(out=t[:], in_=dd["ONES1"][:])
        sb["ONES1"] = t
        epst = pool.tile([128, 1], F32, tag="eps")
        nc.vector.memset(epst[:], EPS)
        sb["eps"] = epst
        e30 = pool.tile([128, 1], F32, tag="eps30")
        nc.vector.memset(e30[:], 1e-30)
        sb["eps30"] = e30
        n8 = pool.tile([128, 1], F32, tag="neg8")
        nc.vector.memset(n8[:], -8.0)
        sb["neg8"] = n8
        self.sb = sb
        # warm the PE transpose path (single sync wait on LDWEIGHTS)
        junk = pool.tile([128, 1], F32, tag="junk")
        wf = self.pps.tile([128, 128], F32, tag="ps", name="warmf")
        nc.tensor.transpose(wf[:], sb["IDF"][:], sb["IDF"][:])
        nc.scalar.copy(out=junk[:, 0:1], in_=wf[:, 0:1])

    def hslice(self, T, h, cols):
        return _opt(T[(h % 4) * DK:(h % 4 + 1) * DK, h // 4, cols])

    def proj_T(self, xT, wname, bname, pool, tag):
        """out[do, s] = W @ x.T + b : [128, 2, 1024] bf16."""
        nc = self.nc
        W = self.sb[wname]
        out = pool.tile([128, 2, SEQ], BF16, tag=tag)
        for dh in range(2):
            for sc in range(2):
                ps = self.pst([128, 512])
                for ih in range(2):
                    nc.tensor.matmul(
                        ps[:], _opt(W[:, ih, dh * 128:(dh + 1) * 128]),
                        _opt(xT[:, ih, sc * 512:(sc + 1) * 512]),
                        start=(ih == 0), stop=(ih == 1))
                nc.scalar.activation(
                    out=_opt(out[:, dh, sc * 512:(sc + 1) * 512]), in_=ps[:],
                    func=AF.Identity, bias=self.sb[bname][:, dh:dh + 1],
                    scale=1.0)
        return out

    def proj_V(self, xT, wname, bname, pool, tag):
        """V natural with ones column: [8][128, H, 33] bf16."""
        nc = self.nc
        W = self.sb[wname]
        bias = self.sb[bname]
        tiles = []
        for st in range(NQT):
            ps = self.pst([128, D])
            for ih in range(2):
                nc.tensor.matmul(ps[:],
                                 _opt(xT[:, ih, st * 128:(st + 1) * 128]),
                                 _opt(W[:, ih, :]),
                                 start=(ih == 0), stop=(ih == 1))
            v = pool.tile([128, H, DK + 1], F16, tag=f"{tag}{st}")
            nc.vector.tensor_tensor(
                out=v[:, :, 0:DK],
                in0=ps[:].rearrange("p (h d) -> p h d", h=H),
                in1=bias[:].rearrange("p (h d) -> p h d", h=H), op=OP.add)
            nc.vector.memset(_opt(v[:, :, DK:DK + 1]), 1.0)
            tiles.append(v)
        return tiles

    # ---------------------------------------------- decay (shared L1/2/3)
    def decay_av(self, lay, qt, hg, G, t_srcs, V, at, wts_pool, oq):
        """From scaled u (G['u']) -> eff -> t -> w -> wt -> o -> at slice.

        G['u']: [128, HG*(B+nm)] bf16, band cols then per-head coarse cols.
        t_srcs: (band_ap(i,h), coarse_ap(i,h)) callables for the score
        factor multiplying eff.
        """
        nc, sb = self.nc, self.sb
        Kt = (qt + 1) * 128
        B = BAND
        C = Kt - B
        nm = C // CF
        u = G["u"]
        # ln -> dg -> eff, all served by natural_log_exp_and_others; the
        # +1e-30 ln bias floors u so ln never emits -inf
        L = G["L"]
        nc.scalar.activation(out=L[:], in_=u[:], func=AF.Ln,
                             bias=sb["eps30"][:])
        nc.scalar.activation(out=L[:], in_=L[:], func=AF.Exp, scale=0.5)
        nc.scalar.activation(out=L[:], in_=L[:], func=AF.Exp, scale=-1.0)
        # t = eff * score-factor: band on vector (2x), coarse on pool
        t = G["t"]
        band_src, coarse_src = t_srcs
        for i in range(HG):
            h = hg * HG + i
            nc.vector.tensor_tensor(
                out=_opt(t[:, i, C:Kt]),
                in0=_opt(L[:, i * B:(i + 1) * B]),
                in1=band_src(i, h), op=OP.mult)
            if nm:
                nc.gpsimd.tensor_tensor(
                    out=_opt(t[:, i, 0:C]),
                    in0=_ap(L, HG * B + i * nm, [[1, nm], [0, CF]]),
                    in1=coarse_src(i, h), op=OP.mult)
        # exp(t - 8): keeps w in f16 range (t can reach ~15 near the diag);
        # the uniform e^-8 cancels in the ones-column normalization
        w = G["w"]
        nc.scalar.activation(out=w[:], in_=t[:], func=AF.Exp,
                             bias=sb["neg8"][:])
        # wt via one batched DMA xbar transpose per head-group
        o = oq.tile([128, HG, DK + 1], F32, tag="o", name="o")
        nblk = qt + 1
        wt = wts_pool.tile([128, HG * nblk, 128], F16, tag="wt", name="wt")
        nc.sync.dma_start_transpose(
            out=wt[:], in_=_opt(w[:].rearrange("p i k -> p (i k)")))
        for i in range(HG):
            h = hg * HG + i
            for kb in range(nblk):
                nc.tensor.matmul(
                    _opt(o[:, i, :]), _opt(wt[:, i * nblk + kb, :]),
                    _opt(V[kb][:, h, :]),
                    start=(kb == 0), stop=(kb == qt), skip_group_check=True)
        # normalize -> at (bf16)
        Wg = G["sml"].tile([128, HG], F32, tag="Wg", name="Wg")
        rW = G["sml"].tile([128, HG], F32, tag="rW", name="rW")
        nc.vector.tensor_scalar_max(out=Wg[:], in0=_opt(o[:, :, DK:DK + 1]),
                                    scalar1=1e-30)
        nc.vector.reciprocal(out=rW[:], in_=Wg[:])
        nc.vector.tensor_tensor(
            out=_opt(at[:, hg * HG:hg * HG + HG, :]),
            in0=_opt(o[:, :, 0:DK]), in1=_bc(rW[:], DK), op=OP.mult)

    # ------------------------------------------ streamed out proj + LN
    def proj_ln_st(self, lay, at, st, res_tiles, spool, dram_out=None):
        """Per-q-tile: transpose at -> out proj -> +res -> LN -> ho (bf16)."""
        nc, sb = self.nc, self.sb
        atT = spool.tile([128, 2, 128], BF16, tag="atT", name="atT")
        nc.sync.dma_start_transpose(
            out=atT[:], in_=_opt(at[:].rearrange("p h d -> p (h d)")))
        W = sb[f"WT_o{lay}"]
        ps = self.pst([128, D])
        for ih in range(2):
            nc.tensor.matmul(ps[:], _opt(atT[:, ih, :]),
                             _opt(W[:, ih, :]), start=(ih == 0),
                             stop=(ih == 1))
        res = res_tiles[st] if isinstance(res_tiles, list) else res_tiles
        x = spool.tile([128, D], F32, tag="lnx")
        nc.vector.tensor_tensor(out=x[:], in0=ps[:], in1=res[:], op=OP.add)
        stats = spool.tile([128, 6], F32, tag="bnst")
        mv = spool.tile([128, 2], F32, tag="bnmv")
        nc.vector.bn_stats(out=stats[:], in_=x[:])
        nc.vector.bn_aggr(out=mv[:], in_=stats[:])
        lv = spool.tile([128, 1], F32, tag="lv")
        nc.scalar.activation(out=lv[:], in_=_opt(mv[:, 1:2]), func=AF.Ln,
                             bias=sb["eps"][:], scale=1.0)
        rstd = spool.tile([128, 1], F32, tag="rstd")
        nc.scalar.activation(out=rstd[:], in_=lv[:], func=AF.Exp,
                             scale=-0.5)
        xn = spool.tile([128, D], F32, tag="lnxn")
        nc.vector.tensor_scalar(
            out=xn[:], in0=x[:], scalar1=_opt(mv[:, 0:1]), scalar2=rstd[:],
            op0=OP.subtract, op1=OP.mult)
        ho = spool.tile([128, D], BF16, tag="ho", name="ho")
        nc.vector.tensor_tensor(out=ho[:], in0=xn[:],
                                in1=sb[f"lng{lay}_r"][:], op=OP.mult)
        nc.vector.tensor_tensor(out=ho[:], in0=ho[:],
                                in1=sb[f"lnb{lay}_r"][:], op=OP.add)
        if dram_out is not None:
            o32 = spool.tile([128, D], F32, tag="ho32", name="ho32")
            nc.vector.tensor_copy(out=o32[:], in_=ho[:])
            nc.sync.dma_start(out=dram_out[st * 128:(st + 1) * 128, :],
                              in_=o32[:])
        return ho

    # ---------------------------------------------- attention (layers 1/2)
    def attention(self, lay, QT, KT, V, res_tiles, spool, hT=None,
                  dram_out=None, dbg=None):
        nc, sb, tc = self.nc, self.sb, self.tc
        with contextlib.ExitStack() as actx:
            zq = actx.enter_context(
                tc.tile_pool(name=f"zq{lay}", bufs=2, space="PSUM"))
            oq = actx.enter_context(
                tc.tile_pool(name=f"oq{lay}", bufs=2, space="PSUM"))
            big = actx.enter_context(tc.tile_pool(name=f"big{lay}", bufs=2))
            mid = actx.enter_context(tc.tile_pool(name=f"mid{lay}", bufs=2))
            sml = actx.enter_context(tc.tile_pool(name=f"sml{lay}", bufs=3))
            wts = actx.enter_context(tc.tile_pool(name=f"wts{lay}", bufs=3))

            hs = []
            for qt in range(NQT):
                Kt = (qt + 1) * 128
                B = BAND
                C = Kt - B
                nm = C // CF
                at = sml.tile([128, H, DK], BF16, tag="at", name="at")
                for hg in range(NG):
                    G = {"sml": sml}
                    # zsC: off-diag cols [0, C); zsB: diag block (the band)
                    zsC = big.tile([128, HG, C + 2] if C else [128, 2, 2],
                                   F16, tag="zsC", name="zsC")
                    zsB = mid.tile([128, HG * B], F16, tag="zsB", name="zsB")
                    for i in range(HG):
                        h = hg * HG + i
                        z = zq.tile([128, Kt], F32, tag="z", name="z")
                        lhs = self.hslice(QT, h,
                                          slice(qt * 128, qt * 128 + 128))
                        tp = ((h % 4) * DK, 0)
                        for ci in range((Kt + 511) // 512):
                            kc = ci * 512
                            cl = min(512, Kt - kc)
                            nc.tensor.matmul(
                                _opt(z[:, kc:kc + cl]), lhs,
                                self.hslice(KT, h, slice(kc, kc + cl)),
                                start=True, stop=True, tile_position=tp,
                                skip_group_check=True)
                        if C:
                            if i % 2 == 0:
                                nc.vector.tensor_scalar_mul(
                                    out=_opt(zsC[:, i, 0:C]),
                                    in0=_opt(z[:, 0:C]), scalar1=ISQ)
                            else:
                                nc.scalar.mul(
                                    out=_opt(zsC[:, i, 0:C]),
                                    in_=_opt(z[:, 0:C]), mul=ISQ)
                        nc.vector.scalar_tensor_tensor(
                            out=_opt(zsB[:, i * B:(i + 1) * B]),
                            in0=_opt(z[:, C:Kt]), scalar=ISQ,
                            in1=sb["M0s"][:], op0=OP.mult, op1=OP.add)
                    # e
                    eB = mid.tile([128, HG * B], BF16, tag="eB", name="eB")
                    nc.scalar.activation(out=eB[:], in_=zsB[:], func=AF.Exp)
                    if C:
                        eC = big.tile([128, HG, C + 2], BF16, tag="eC",
                                      name="eC")
                        nc.scalar.activation(
                            out=_opt(eC[:, :, 0:C]), in_=_opt(zsC[:, :, 0:C]),
                            func=AF.Exp)
                        nc.vector.memset(
                            _ap(eC, C, [[C + 2, HG], [1, 2]]), 0.0)
                    # band: segmented reversed scan over eB directly
                    Sb = mid.tile([128, HG * B + 1], BF16, tag="Sb",
                                  name="Sb")
                    nc.vector.memset(_opt(Sb[:, HG * B:HG * B + 1]), 0.0)
                    nc.vector.tensor_tensor_scan(
                        out=_rev(Sb[:, 0:HG * B]),
                        data0=_rev(sb["SEGB1"][:, 0:HG * B]),
                        data1=_rev(eB[:]),
                        initial=0.0, op0=OP.mult, op1=OP.add)
                    bm = sml.tile([128, HG], F32, tag="bm", name="bm")
                    nc.vector.tensor_copy(out=bm[:],
                                          in_=_ap(Sb, 0, [[B, HG]]))
                    # coarse: shifted pair-sums (vector 2x) + strided pool
                    if nm:
                        eCf = eC[:].rearrange("p i c -> p (i c)")
                        CW = C + 2
                        s2 = big.tile([128, HG * CW], BF16, tag="s2",
                                      name="s2")
                        nc.vector.tensor_tensor(
                            out=_opt(s2[:, 0:HG * CW - 1]),
                            in0=_opt(eCf[:, 0:HG * CW - 1]),
                            in1=_opt(eCf[:, 1:HG * CW]), op=OP.add)
                        b4 = mid.tile([128, HG, C // 4], BF16, tag="b4",
                                      name="b4")
                        nc.gpsimd.tensor_tensor(
                            out=b4[:], in0=_ap(s2, 0, [[CW, HG], [4, C // 4]]),
                            in1=_ap(s2, 2, [[CW, HG], [4, C // 4]]),
                            op=OP.add)
                        b8 = mid.tile([128, HG, C // 8], BF16, tag="b8",
                                      name="b8")
                        nc.gpsimd.tensor_tensor(
                            out=b8[:],
                            in0=_ap(b4, 0, [[C // 4, HG], [2, C // 8]]),
                            in1=_ap(b4, 1, [[C // 4, HG], [2, C // 8]]),
                            op=OP.add)
                        bx = mid.tile([128, HG, nm + 1], F32, tag="bx",
                                      name="bx")
                        nc.gpsimd.tensor_tensor(
                            out=_opt(bx[:, :, 0:nm]),
                            in0=_ap(b8, 0, [[C // 8, HG], [2, nm]]),
                            in1=_ap(b8, 1, [[C // 8, HG], [2, nm]]),
                            op=OP.add)
                        nc.vector.tensor_copy(
                            out=_ap(bx, nm, [[nm + 1, HG], [1, 1]]),
                            in_=_bc(bm[:], 1))
                        SBi = mid.tile([128, HG * (nm + 1)], F32, tag="SBi",
                                       name="SBi")
                        nc.vector.tensor_tensor_scan(
                            out=_rev(SBi[:]),
                            data0=_rev(sb[f"SEGC{qt}"][:]),
                            data1=_rev(_opt(bx[:].rearrange(
                                "p i m -> p (i m)"))),
                            initial=0.0, op0=OP.mult, op1=OP.add)
                        E = sml.tile([128, HG], F32, tag="E", name="E")
                        nc.vector.tensor_copy(out=E[:],
                                              in_=_ap(SBi, 0, [[nm + 1, HG]]))
                    else:
                        E = bm
                    # rEg = gamma^2 / E
                    rEg = sml.tile([128, HG], F32, tag="rEg", name="rEg")
                    nc.vector.reciprocal(out=rEg[:], in_=E[:])
                    nc.vector.tensor_tensor(
                        out=rEg[:], in0=rEg[:],
                        in1=_opt(sb[f"gam2_{lay}"][:, hg * HG:hg * HG + HG]),
                        op=OP.mult)
                    # u = S * pos * rEg (band + coarse super-tile)
                    u = mid.tile([128, HG * (B + nm)], BF16, tag="u",
                                 name="u")
                    nc.vector.tensor_tensor(
                        out=_opt(u[:, 0:HG * B]), in0=_opt(Sb[:, 1:HG * B + 1]),
                        in1=sb["POSB41"][:], op=OP.mult)
                    if nm:
                        nc.vector.tensor_tensor(
                            out=_ap(u, HG * B, [[nm, HG], [1, nm]]),
                            in0=_ap(SBi, 1, [[nm + 1, HG], [1, nm]]),
                            in1=_ap(sb["POSCR"], 64 - 8 * qt,
                                    [[0, HG], [1, nm]]),
                            op=OP.mult)
                    for i in range(HG):
                        nc.vector.tensor_scalar_mul(
                            out=_opt(u[:, i * B:(i + 1) * B]),
                            in0=_opt(u[:, i * B:(i + 1) * B]),
                            scalar1=rEg[:, i:i + 1])
                        if nm:
                            nc.vector.tensor_scalar_mul(
                                out=_opt(u[:, HG * B + i * nm:
                                           HG * B + (i + 1) * nm]),
                                in0=_opt(u[:, HG * B + i * nm:
                                           HG * B + (i + 1) * nm]),
                                scalar1=rEg[:, i:i + 1])
                    G["u"] = u
                    G["L"] = mid.tile([128, HG * (B + nm)], F16, tag="L",
                                      name="L")
                    G["t"] = big.tile([128, HG, Kt], F16, tag="t", name="t")
                    G["w"] = big.tile([128, HG, Kt], F16, tag="w", name="w")

                    def band_src(i, h, zsB=zsB, B=B):
                        return _opt(zsB[:, i * B:(i + 1) * B])

                    def coarse_src(i, h, zsC=zsC, C=C):
                        return _opt(zsC[:, i, 0:C])
                    self.decay_av(lay, qt, hg, G, (band_src, coarse_src),
                                  V, at, wts, oq)
                if dbg is not None:
                    o32 = spool.tile([128, D], F32, tag="dbgat")
                    nc.vector.tensor_copy(
                        out=o32[:].rearrange("p (h d) -> p h d", h=H),
                        in_=at[:])
                    nc.sync.dma_start(out=dbg[qt * 128:(qt + 1) * 128, :],
                                      in_=o32[:])
                ho = self.proj_ln_st(lay, at, qt, res_tiles, spool, dram_out)
                if hT is not None:
                    nc.sync.dma_start_transpose(
                        out=hT[:, :, qt * 128:(qt + 1) * 128], in_=ho[:])
                hs.append(ho)
            return hs

    # ---------------------------------------------- layer-3 attention
    def attention3(self, V, rows, dd, cum3pad, fpool, spool, al_dram,
                   out_dram, mode="full", dbg=None, dram_out_hh=None):
        nc, sb, tc = self.nc, self.sb, self.tc
        # final-mixture consts (loaded here so they don't occupy SBUF
        # during layers 1/2)
        t = fpool.tile([128, 2, H * D], BF16, tag="Wblk")
        nc.sync.dma_start(
            out=t[:], in_=dd["Wblk"][:].rearrange("(a p) d -> p a d", p=128))
        sb["Wblk"] = t
        t = fpool.tile([1, H * D], BF16, tag="blv8")
        nc.sync.dma_start(out=t[:], in_=dd["blv8"][:])
        sb["blv8"] = t
        # partition-broadcast loads of the score/cumsum rows
        d_c3, d_nc = rows
        c3bc = fpool.tile([128, H, SEQ], F16, tag="c3bc")
        dap = d_c3[:]
        nc.sync.dma_start(out=c3bc[:], in_=bass.AP(
            tensor=dap.tensor, offset=0,
            ap=[[0, 128], [SEQ, H], [1, SEQ]]))
        ncum = fpool.tile([128, H, SEQ], F32, tag="ncum")
        dap = d_nc[:]
        nc.sync.dma_start(out=ncum[:], in_=bass.AP(
            tensor=dap.tensor, offset=0,
            ap=[[0, 128], [SEQ, H], [1, SEQ]]))
        hhTf = fpool.tile([128, 2, SEQ], BF16, tag="hhTf")
        with contextlib.ExitStack() as actx:
            oq = actx.enter_context(
                tc.tile_pool(name="oq3", bufs=2, space="PSUM"))
            big = actx.enter_context(tc.tile_pool(name="big3", bufs=2))
            mid = actx.enter_context(tc.tile_pool(name="mid3", bufs=2))
            sml = actx.enter_context(tc.tile_pool(name="sml3", bufs=3))
            wts = actx.enter_context(tc.tile_pool(name="wts3", bufs=2))

            for qt in range(NQT):
                Kt = (qt + 1) * 128
                B = BAND
                C = Kt - B
                nm = C // CF
                at = sml.tile([128, H, DK], BF16, tag="at3", name="at3")
                # E3 column (strict-causal prefix mass at row q)
                e3ps = self.pst([128, 8])
                nc.tensor.transpose(
                    e3ps[:], _opt(cum3pad[:, qt * 128:qt * 128 + 128]),
                    _opt(sb["IDF"][0:8, 0:8]))
                E3 = sml.tile([128, H], F32, tag="E3", name="E3")
                nc.vector.tensor_scalar_max(out=E3[:], in0=e3ps[:],
                                            scalar1=1e-30)
                rE3g = sml.tile([128, H], F32, tag="rE3g", name="rE3g")
                nc.vector.reciprocal(out=rE3g[:], in_=E3[:])
                nc.vector.tensor_tensor(out=rE3g[:], in0=rE3g[:],
                                        in1=sb["gam2_3"][:], op=OP.mult)
                for hg in range(NG):
                    G = {"sml": sml}
                    u = mid.tile([128, HG * (B + nm)], BF16, tag="u3",
                                 name="u3")
                    for i in range(HG):
                        h = hg * HG + i
                        # u = max(E3 + ncum, 0) * pos * rE3g, all on vector
                        nc.vector.scalar_tensor_tensor(
                            out=_opt(u[:, i * B:(i + 1) * B]),
                            in0=_opt(ncum[:, h, C:Kt]),
                            scalar=_opt(E3[:, h:h + 1]),
                            in1=_opt(sb["POSB41"][:, 0:B]),
                            op0=OP.add, op1=OP.mult)
                        nc.vector.tensor_scalar_mul(
                            out=_opt(u[:, i * B:(i + 1) * B]),
                            in0=_opt(u[:, i * B:(i + 1) * B]),
                            scalar1=rE3g[:, h:h + 1])
                        if nm:
                            nc.vector.scalar_tensor_tensor(
                                out=_ap(u, HG * B + i * nm, [[1, nm]]),
                                in0=_ap(ncum, (CF - 1) + h * SEQ,
                                        [[CF, nm]]),
                                scalar=_opt(E3[:, h:h + 1]),
                                in1=_ap(sb["POSCR"], 64 - 8 * qt, [[1, nm]]),
                                op0=OP.add, op1=OP.mult)
                            nc.vector.tensor_scalar_mul(
                                out=_ap(u, HG * B + i * nm, [[1, nm]]),
                                in0=_ap(u, HG * B + i * nm, [[1, nm]]),
                                scalar1=rE3g[:, h:h + 1])
                    G["u"] = u
                    G["L"] = mid.tile([128, HG * (B + nm)], F16, tag="L3",
                                      name="L3")
                    G["t"] = big.tile([128, HG, Kt], F16, tag="t3", name="t3")
                    G["w"] = big.tile([128, HG, Kt], F16, tag="w3", name="w3")
                    # c3 band (= diag block) with the strict mask
                    ccomp = mid.tile([128, HG, B], F16, tag="ccmp",
                                     name="ccmp")
                    for i in range(HG):
                        h = hg * HG + i
                        nc.vector.tensor_tensor(
                            out=_opt(ccomp[:, i, :]),
                            in0=_opt(c3bc[:, h, Kt - 128:Kt]),
                            in1=sb["M3"][:], op=OP.add)

                    def band_src(i, h, ccomp=ccomp):
                        return _opt(ccomp[:, i, :])

                    def coarse_src(i, h, c3bc=c3bc, C=C):
                        return _opt(c3bc[:, h, 0:C])
                    self.decay_av(3, qt, hg, G, (band_src, coarse_src),
                                  V, at, wts, oq)
                if dbg is not None:
                    o32 = spool.tile([128, D], F32, tag="dbgat3")
                    nc.vector.tensor_copy(
                        out=o32[:].rearrange("p (h d) -> p h d", h=H),
                        in_=at[:])
                    nc.sync.dma_start(out=dbg[qt * 128:(qt + 1) * 128, :],
                                      in_=o32[:])
                if mode == "a3":
                    o32 = spool.tile([128, D], F32, tag="s4o")
                    nc.vector.tensor_copy(
                        out=o32[:].rearrange("p (h d) -> p h d", h=H),
                        in_=at[:])
                    nc.sync.dma_start(
                        out=out_dram[qt * 128:(qt + 1) * 128, :], in_=o32[:])
                    continue
                ho = self.proj_ln_st(3, at, qt, sb["knowr_r"], spool,
                                     dram_out_hh)
                if mode == "hh":
                    o32 = spool.tile([128, D], F32, tag="s5o")
                    nc.vector.tensor_copy(out=o32[:], in_=ho[:])
                    nc.sync.dma_start(
                        out=out_dram[qt * 128:(qt + 1) * 128, :], in_=o32[:])
                    continue
                nc.sync.dma_start_transpose(
                    out=hhTf[:, :, qt * 128:(qt + 1) * 128], in_=ho[:])
            # final mixture deferred past the qt loop: keeps Sigmoid to a
            # single ACT-table swap instead of two per q-tile
            if mode == "full":
                for st in range(NQT):
                    self.final_st(st, hhTf, al_dram, out_dram, spool)

    # ------------------------------------------------ layer-3 prologue
    def l3_rows(self, h1T, mpool, lpool, dram_rows):
        """Compute the layer-3 score/cumsum rows -> DRAM (broadcast loads
        happen chunked inside attention3). Returns cum3pad (lpool)."""
        nc, sb = self.nc, self.sb
        KT3 = self.proj_T(h1T, "WT_k3", "bk3_c", mpool, tag="KT3")
        c3 = mpool.tile([8, SEQ], F32, tag="c3")
        for scc in range(2):
            ps = self.pst([8, 512])
            for ih in range(2):
                nc.tensor.matmul(ps[:], _opt(sb["q3blk"][:, ih, :]),
                                 _opt(KT3[:, ih, scc * 512:(scc + 1) * 512]),
                                 start=(ih == 0), stop=(ih == 1))
            nc.vector.tensor_scalar_mul(
                out=_opt(c3[:, scc * 512:(scc + 1) * 512]), in0=ps[:],
                scalar1=ISQ)
        e3 = mpool.tile([8, SEQ], F32, tag="e3")
        nc.scalar.activation(out=e3[:], in_=c3[:], func=AF.Exp)
        cum3pad = lpool.tile([8, SEQ + 128], F32, tag="cum3pad")
        nc.vector.memset(_opt(cum3pad[:, 0:1]), 0.0)
        nc.vector.tensor_tensor_scan(
            out=_opt(cum3pad[:, 1:SEQ + 1]), data0=e3[:], data1=e3[:],
            initial=0.0, op0=OP.add, op1=OP.bypass)
        nc.vector.memset(_opt(cum3pad[:, SEQ + 1:]), 0.0)
        # rows -> DRAM; attention3 loads them partition-broadcast
        c3b = mpool.tile([8, SEQ], F16, tag="c3b")
        nc.vector.tensor_copy(out=c3b[:], in_=c3[:])
        ncm = mpool.tile([8, SEQ], F32, tag="ncm")
        nc.vector.tensor_scalar_mul(out=ncm[:], in0=_opt(cum3pad[:, 1:SEQ + 1]),
                                    scalar1=-1.0)
        d_c3, d_nc = dram_rows
        nc.sync.dma_start(out=d_c3[:], in_=c3b[:])
        nc.sync.dma_start(out=d_nc[:], in_=ncm[:])
        return cum3pad

    # ------------------------------------------------ final mixture
    def final_st(self, st, hhT, al_dram, out_dram, spool):
        """Block-diagonal Wlv matmul: val = sigmoid(hh @ Wblk + blv8)."""
        nc, sb = self.nc, self.sb
        al = spool.tile([128, H], F32, tag="al", name="al")
        nc.sync.dma_start(out=al[:],
                          in_=al_dram[st * 128:(st + 1) * 128, :])
        val = spool.tile([128, H, D], BF16, tag="val", name="val")
        for j in range(4):
            vps = self.pst([128, 512])
            for ih in range(2):
                nc.tensor.matmul(
                    vps[:], _opt(hhT[:, ih, st * 128:(st + 1) * 128]),
                    _opt(sb["Wblk"][:, ih, j * 512:(j + 1) * 512]),
                    start=(ih == 0), stop=False)
            nc.tensor.matmul(
                vps[:], sb["ONES1"][:],
                _opt(sb["blv8"][0:1, j * 512:(j + 1) * 512]),
                start=False, stop=True)
            nc.scalar.activation(out=_ap(val, j * 512, [[1, 512]]),
                                 in_=vps[:], func=AF.Sigmoid)
        acc = spool.tile([128, D], F32, tag="facc", name="facc")
        nc.vector.tensor_scalar_mul(
            out=acc[:], in0=_opt(val[:, 0, :]), scalar1=al[:, 0:1])
        for h in range(1, H):
            nc.vector.scalar_tensor_tensor(
                out=acc[:], in0=_opt(val[:, h, :]), scalar=al[:, h:h + 1],
                in1=acc[:], op0=OP.mult, op1=OP.add)
        nc.sync.dma_start(out=out_dram[st * 128:(st + 1) * 128, :],
                          in_=acc[:])


class _StageDoneExc(Exception):
    pass


_StageDone = _StageDoneExc()


def _patched_act_tables(nc):
    import types
    from concourse.hw_specs import get_activation_tables
    import concourse.bass_primitives_rust as _bpr

    def patched(self):
        has_act = any(isinstance(i, mybir.InstActivation)
                      for b in self.main_func.blocks
                      for i in b.instructions)
        if not has_act:
            return
        tables = []
        for name, fns in get_activation_tables(self.m.arch).items():
            if name in ("exp_and_others", "natural_log", "exp_and_friends"):
                fns = set()
            tables.append((name, fns))
        from concourse import bacc as _bacc
        _bacc._bass_rust.insert_act_table_loads(self, tables)

    nc.insert_act_table_loads = types.MethodType(patched, nc)


def build(derived, debug=False, stage=None):
    stage = stage or os.environ.get("V2_STAGE", "full")
    nc = bacc.Bacc(None, target_bir_lowering=False)
    _patched_act_tables(nc)
    dd = {}
    for name, arr in derived.items():
        if name.startswith("_"):
            continue
        dt = {np.dtype(np.float32): F32, np.dtype(bf16): BF16,
              np.dtype(f16): F16}[np.dtype(arr.dtype)]
        dd[name] = nc.dram_tensor(name, list(arr.shape), dt,
                                  kind="ExternalInput")
    for nm, shape, dt in (("xbf_q", [SEQ, D], BF16), ("xbf_s", [SEQ, D], BF16),
                          ("xr_q", [SEQ, D], F32), ("xr_s", [SEQ, D], F32),
                          ("al_nat", [SEQ, H], F32)):
        dd[nm] = nc.dram_tensor(nm, shape, dt, kind="ExternalInput")
    out = nc.dram_tensor("out", [SEQ, D], F32, kind="ExternalOutput")
    d_c3 = nc.dram_tensor("rows_c3", [8, SEQ], F16, kind="Internal")
    d_nc = nc.dram_tensor("rows_nc", [8, SEQ], F32, kind="Internal")

    def dbg(name):
        return nc.dram_tensor(name, [SEQ, D], F32,
                              kind="ExternalOutput") if debug else None

    with tile.TileContext(nc) as tc, contextlib.ExitStack() as ctx:
      try:
        kb = KB(nc, tc, ctx)
        kb.pps = ctx.enter_context(
            tc.tile_pool(name="pps", bufs=2, space="PSUM"))
        kb.load_consts(dd)
        lpool = ctx.enter_context(tc.tile_pool(name="l3pool", bufs=1))

        with tc.tile_pool(name="p2", bufs=1) as p2:
            # -------- prologue: transposes + L1 AND L2 projections --------
            with tc.tile_pool(name="p1", bufs=1) as p1:
                with tc.tile_pool(name="xt", bufs=1) as xt:
                    xT_q = xt.tile([128, 2, SEQ], BF16, tag="xTq")
                    nc.sync.dma_start_transpose(out=xT_q[:],
                                                in_=dd["xbf_q"][:])
                    xT_s = xt.tile([128, 2, SEQ], BF16, tag="xTs")
                    nc.sync.dma_start_transpose(out=xT_s[:],
                                                in_=dd["xbf_s"][:])
                    QT1 = kb.proj_T(xT_q, "WT_q1", "bq1_c", p1, tag="QT1")
                    V1 = kb.proj_V(xT_q, "WT_v1", "bv1_r", p1, tag="V1")
                    QT2 = kb.proj_T(xT_s, "WT_q2", "bq2_c", p2, tag="QT2")
                    V2 = kb.proj_V(xT_s, "WT_v2", "bv2_r", p2, tag="V2")

                # ---------------- layer 1 (on x_q) ----------------
                with tc.tile_pool(name="r1", bufs=1) as r1, \
                        tc.tile_pool(name="r1s", bufs=2) as r1s:
                    xr1 = []
                    for st in range(NQT):
                        t = r1.tile([128, D], F32, tag=f"xr1_{st}")
                        nc.sync.dma_start(
                            out=t[:],
                            in_=dd["xr_q"][st * 128:(st + 1) * 128, :])
                        xr1.append(t)
                    h1T = r1.tile([128, 2, SEQ], BF16, tag="h1T")
                    kb.attention(
                        1, QT1, QT1, V1, xr1, r1s, hT=h1T,
                        dram_out=(out if stage == "l1" else dbg("dbg_h1")),
                        dbg=(nc.dram_tensor("dbg_attn1", [SEQ, D], F32,
                                            kind="ExternalOutput")
                             if debug else None))
                    if stage == "l1":
                        raise _StageDone
                    # layer-3 prologue rows (uses h1T) - overlaps layer 2
                    with tc.tile_pool(name="l3r", bufs=1) as l3r:
                        cum3pad = kb.l3_rows(h1T, l3r, lpool, (d_c3, d_nc))
            # ---------------- layer 2 (on x_s) ----------------
            with tc.tile_pool(name="r2", bufs=1) as r2, \
                    tc.tile_pool(name="r2s", bufs=2) as r2s:
                xr2 = []
                for st in range(NQT):
                    t = r2.tile([128, D], F32, tag=f"xr2_{st}")
                    nc.sync.dma_start(
                        out=t[:], in_=dd["xr_s"][st * 128:(st + 1) * 128, :])
                    xr2.append(t)
                h2T = r2.tile([128, 2, SEQ], BF16, tag="h2T")
                kb.attention(
                    2, QT2, QT2, V2, xr2, r2s, hT=h2T,
                    dram_out=(out if stage == "l2" else dbg("dbg_h2")))
                V3 = kb.proj_V(h2T, "WT_v3", "bv3_r", lpool, tag="V3")
                if stage == "pro":
                    for st in range(NQT):
                        o32 = r2s.tile([128, D], F32, tag="s3o")
                        nc.vector.tensor_copy(
                            out=o32[:].rearrange("p (h d) -> p h d", h=H),
                            in_=V3[st][:, :, 0:DK])
                        nc.sync.dma_start(
                            out=out[st * 128:(st + 1) * 128, :], in_=o32[:])
        if stage in ("l2", "pro"):
            raise _StageDone
        # ---------------- layer 3 + final ----------------
        with tc.tile_pool(name="l3f", bufs=1) as l3f, \
                tc.tile_pool(name="r3s", bufs=3) as r3s:
            mode = stage if stage in ("a3", "hh") else "full"
            kb.attention3(V3, (d_c3, d_nc), dd, cum3pad, l3f, r3s,
                          dd["al_nat"], out, mode=mode,
                          dbg=(nc.dram_tensor("dbg_attn3", [SEQ, D], F32,
                                              kind="ExternalOutput")
                               if debug else None),
                          dram_out_hh=dbg("dbg_hh"))
      except _StageDoneExc:
        pass
    nc.compile()
    return nc


_CACHE = {}


def kernel(**inputs):
    drv = host_prep(inputs)
    if "nc" not in _CACHE:
        _CACHE["nc"] = build(drv)
    nc = _CACHE["nc"]
    in_maps = per_batch_maps(inputs, drv)
    from concourse.bass_utils import run_bass_kernel_spmd
    res = run_bass_kernel_spmd(nc, in_maps, core_ids=list(range(BS)))
    out = np.stack([np.asarray(res.results[b]["out"]) for b in range(BS)],
                   axis=0)
    return out.astype(np.float32)


if __name__ == "__main__":
    print("kernel module loaded OK")



# revision 50
# speedup vs baseline: 1.2657x; 1.2657x over previous
"""Trainium2 Bass kernel for nn_DTransformer (sparse attention w/ distance decay).

Sharding: data-parallel over batch (bs=8 -> 8 cores, weights replicated).

v2 design notes:
- All PE matmuls in bf16 (fp32 is 4 cyc/row on the PE; bf16 is 1).
- Causal mask folded into the zs write (STT on the diag block) - no PE mask
  matmuls; no rank-1 bias matmuls (residuals pre-biased on host, alpha of the
  final mixture computed on host, blv added on vector).
- A@V weight transposes done by the DMA xbar (dma_start_transpose), one
  descriptor per (head, q-tile), not by the PE.
- Distance-decay eff computed at full resolution only in a 256-wide diagonal
  band; beyond that on a stride-16 coarse grid (validated: 3e-5 output err).
  Suffix masses: segmented reversed scan over a compacted band (poison-mask
  trick resets at head boundaries) chained with a tiny scan over coarse block
  sums built by pairwise adds on the (otherwise idle) GPSIMD engine.
- sqrt via exp(0.5*ln(u)): ln+exp live in one ACT table set, so the scalar
  engine never swaps tables inside the hot loop (sqrt would force 2 swaps per
  q-tile). LayerNorm's 1/std likewise uses exp(-0.5*ln(var+eps)).
- zs / t / block-sum elementwise passes run on GPSIMD (Pool) to unload the
  vector engine.
"""

import os
import sys
import contextlib

for _p in ("/opt/trn_rl_repo", "/root/.axon_site/_ro/trn_rl_repo"):
    if os.path.isdir(_p) and _p not in sys.path:
        sys.path.insert(0, _p)

import numpy as np
import ml_dtypes

import concourse.bass as bass
import concourse.mybir as mybir
import concourse.tile as tile
from concourse import bacc

F32 = mybir.dt.float32
F16 = mybir.dt.float16
BF16 = mybir.dt.bfloat16
AF = mybir.ActivationFunctionType
OP = mybir.AluOpType

D = 256
H = 8
HG = 4            # heads per group
NG = H // HG
DK = 32
SEQ = 1024
BS = 8
NQT = SEQ // 128
ISQ = float(1.0 / np.sqrt(np.float32(DK)))
MASKF = -53000.0   # fits f16; exp() underflows to exactly 0
EPS = 1e-5
BAND = 128         # full-res band width (= diag block)
CF = 16            # coarse cell width

bf16 = ml_dtypes.bfloat16
f16 = np.float16
KEEP0 = frozenset({0})


def _opt(ap):
    return ap.opt(keep_dims=KEEP0)


def _rev(ap):
    """Reverse the innermost free dim of an AP (squeeze count-1 dims)."""
    pairs = [list(x) for x in ap.ap]
    keep = [pairs[0]] + [x for x in pairs[1:] if x[1] != 1]
    assert len(keep) == 2, f"need 2D-able ap, got {ap.ap}"
    (ps, pc), (fs, fc) = keep
    return bass.AP(tensor=ap.tensor, offset=ap.offset + fs * (fc - 1),
                   ap=[[ps, pc], [-fs, fc]])


def _ap(t, offset, dims):
    """Build an AP on tile t's tensor with explicit [stride, count] dims."""
    base = t[:]
    return bass.AP(tensor=base.tensor, offset=base.offset + offset,
                   ap=[list(base.ap[0])] + [list(d) for d in dims])


def _bc(ap, n):
    pairs = [list(x) for x in ap.ap]
    return bass.AP(tensor=ap.tensor, offset=ap.offset, ap=pairs + [[0, n]])


# ---------------------------------------------------------------- host prep

def host_prep(inputs):
    g = {k: np.asarray(v) for k, v in inputs.items()}

    def f32(x):
        return np.ascontiguousarray(np.asarray(x, dtype=np.float32))

    def as_bf(x):
        return np.ascontiguousarray(np.asarray(x, np.float32).astype(bf16))

    drv = {}
    for i, names in ((1, ("q", "v", "o")), (2, ("q", "v", "o")),
                     (3, ("k", "v", "o"))):
        for n in names:
            drv[f"WT_{n}{i}"] = as_bf(g[f"W{n}{i}"].T)      # [din, dout]
    for nm in ("bq1", "bq2", "bk3"):
        drv[nm + "_c"] = f32(np.asarray(g[nm], np.float32).reshape(2, 128).T)
    for nm in ("bv1", "bv2", "bv3"):
        drv[nm + "_r"] = f32(g[nm]).reshape(1, D)
    drv["blv_r"] = f32(g["blv"]).reshape(1, D)
    for i in (1, 2, 3):
        drv[f"lng{i}_r"] = f32(g[f"lng{i}"]).reshape(1, D)
        drv[f"lnb{i}_r"] = f32(g[f"lnb{i}"]).reshape(1, D)
        gam = -np.logaddexp(0.0, f32(g[f"g{i}"]).reshape(H))
        drv[f"gam2_{i}"] = f32((gam * gam).reshape(1, H))
    know = f32(g["know"]).reshape(D)
    q3 = know @ f32(g["Wq3"]).T + f32(g["bq3"])
    q3blk = np.zeros((D, H), np.float32)
    for h in range(H):
        q3blk[h * DK:(h + 1) * DK, h] = q3[h * DK:(h + 1) * DK]
    drv["q3blk"] = as_bf(q3blk)
    drv["knowr_r"] = f32(know + f32(g["bo3"])).reshape(1, D)  # pre-biased res
    drv["WlvT"] = as_bf(np.tile(g["Wlv"].T, (4, 1)))          # [128, 256] x4

    p = np.arange(128, dtype=np.float32)[:, None]
    j1 = np.arange(128, dtype=np.float32)[None, :]
    posb1 = np.maximum(p - j1, 0.0)                  # band == diag block
    drv["POSB41"] = np.ascontiguousarray(
        np.tile(posb1, (1, HG)).astype(bf16))        # [128, 4*128]
    # descending coarse pos table: col jj -> 16*(64-jj) + p - 7.5; the
    # per-qt slice [64-8qt : 64-8qt+nm] walks cells m=0..nm-1 ascending
    jj = np.arange(72, dtype=np.float32)[None, :]
    drv["POSCR"] = np.ascontiguousarray(
        (CF * (64.0 - jj) + p - (CF - 1) / 2.0).astype(bf16))  # [128, 72]

    segb1 = np.ones((128, HG * 128), np.float32)
    segb1[:, 127::128] = 0.0
    drv["SEGB1"] = np.ascontiguousarray(segb1.astype(bf16))
    for qt in range(1, NQT):
        nm = qt * 128 // CF
        sc = np.ones((128, HG * (nm + 1)), np.float32)
        sc[:, nm::nm + 1] = 0.0
        drv[f"SEGC{qt}"] = np.ascontiguousarray(sc.astype(bf16))
    drv["ONES4"] = f32(np.ones((128, 128)))

    jj = np.arange(128)[None, :]
    drv["M0s"] = np.ascontiguousarray(
        np.where(jj <= np.arange(128)[:, None], 0.0, MASKF).astype(f16))
    drv["M3"] = np.ascontiguousarray(
        np.where(jj < np.arange(128)[:, None], 0.0, -6e4).astype(f16))
    drv["IDF"] = f32(np.eye(128))

    # host-side alpha for the final mixture (tiny: [bs, s, h])
    kk = know.reshape(H, DK) @ f32(g["Wlk"]).T + f32(g["blk"])
    kk = 1.0 / (1.0 + np.exp(-kk))
    q = np.asarray(g["q_emb"], np.float32)
    beta = np.einsum("hd,bsd->bsh", kk, q)
    beta -= beta.max(-1, keepdims=True)
    ee = np.exp(beta)
    drv["_alpha"] = (ee / ee.sum(-1, keepdims=True)).astype(np.float32)
    return drv


def per_batch_maps(inputs, drv):
    q = np.asarray(inputs["q_emb"], np.float32)
    s = np.asarray(inputs["s_emb"], np.float32)
    bo1 = np.asarray(inputs["bo1"], np.float32)
    bo2 = np.asarray(inputs["bo2"], np.float32)
    alpha = drv["_alpha"]
    base = {k: v for k, v in drv.items() if not k.startswith("_")}
    maps = []
    for b in range(BS):
        m = dict(base)
        m["xbf_q"] = np.ascontiguousarray(q[b].astype(bf16))
        m["xbf_s"] = np.ascontiguousarray(s[b].astype(bf16))
        m["xr_q"] = np.ascontiguousarray(q[b] + bo1)
        m["xr_s"] = np.ascontiguousarray(s[b] + bo2)
        m["al_nat"] = np.ascontiguousarray(alpha[b])
        maps.append(m)
    return maps


# ---------------------------------------------------------------- builder

class KB:
    def __init__(self, nc, tc, ctx):
        self.nc, self.tc, self.ctx = nc, tc, ctx

    def pst(self, shape):
        return self.pps.tile(shape, F32, tag="ps", name="ps")

    def load_consts(self, dd):
        nc = self.nc
        pool = self.ctx.enter_context(self.tc.tile_pool(name="consts", bufs=1))
        sb = {}
        for i, names in ((1, ("q", "v", "o")), (2, ("q", "v", "o")),
                         (3, ("k", "v", "o"))):
            for n in names:
                t = pool.tile([128, 2, D], BF16, tag=f"WT_{n}{i}")
                nc.sync.dma_start(
                    out=t[:],
                    in_=dd[f"WT_{n}{i}"][:].rearrange("(a p) d -> p a d", p=128))
                sb[f"WT_{n}{i}"] = t
        t = pool.tile([128, 2, H], BF16, tag="q3blk")
        nc.sync.dma_start(
            out=t[:], in_=dd["q3blk"][:].rearrange("(a p) h -> p a h", p=128))
        sb["q3blk"] = t
        for nm in list(dd.keys()):
            if nm.startswith(("POSB", "POSC", "SEGB", "SEGC", "M0s", "M3",
                              "IDF", "WlvT")) or nm.endswith("_c"):
                src = dd[nm]
                t = pool.tile(list(src.shape), src.dtype, tag=nm)
                nc.sync.dma_start(out=t[:], in_=src[:])
                sb[nm] = t
        for nm in ("bv1_r", "bv2_r", "bv3_r", "lng1_r", "lng2_r", "lng3_r",
                   "lnb1_r", "lnb2_r", "lnb3_r", "knowr_r", "gam2_1",
                   "gam2_2", "gam2_3", "blv_r"):
            src = dd[nm]
            n = src.shape[1]
            t = pool.tile([128, n], F32, tag=nm)
            nc.sync.dma_start(
                out=t[:],
                in_=bass.AP(tensor=src, offset=0, ap=[[0, 128], [1, n]]))
            sb[nm] = t
        t = pool.tile([128, 128], F32, tag="ONES4")
        nc.sync.dma_start(out=t[:], in_=dd["ONES4"][:])
        sb["ONES4"] = t
        epst = pool.tile([128, 1], F32, tag="eps")
        nc.vector.memset(epst[:], EPS)
        sb["eps"] = epst
        e30 = pool.tile([128, 1], F32, tag="eps30")
        nc.vector.memset(e30[:], 1e-30)
        sb["eps30"] = e30
        self.sb = sb
        # warm the PE transpose path (single sync wait on LDWEIGHTS)
        junk = pool.tile([128, 1], F32, tag="junk")
        wf = self.pps.tile([128, 128], F32, tag="ps", name="warmf")
        nc.tensor.transpose(wf[:], sb["IDF"][:], sb["IDF"][:])
        nc.scalar.copy(out=junk[:, 0:1], in_=wf[:, 0:1])

    def hslice(self, T, h, cols):
        return _opt(T[(h % 4) * DK:(h % 4 + 1) * DK, h // 4, cols])

    def proj_T(self, xT, wname, bname, pool, tag):
        """out[do, s] = W @ x.T + b : [128, 2, 1024] bf16."""
        nc = self.nc
        W = self.sb[wname]
        out = pool.tile([128, 2, SEQ], BF16, tag=tag)
        for dh in range(2):
            for sc in range(2):
                ps = self.pst([128, 512])
                for ih in range(2):
                    nc.tensor.matmul(
                        ps[:], _opt(W[:, ih, dh * 128:(dh + 1) * 128]),
                        _opt(xT[:, ih, sc * 512:(sc + 1) * 512]),
                        start=(ih == 0), stop=(ih == 1))
                nc.scalar.activation(
                    out=_opt(out[:, dh, sc * 512:(sc + 1) * 512]), in_=ps[:],
                    func=AF.Identity, bias=self.sb[bname][:, dh:dh + 1],
                    scale=1.0)
        return out

    def proj_V(self, xT, wname, bname, pool, tag):
        """V natural with ones column: [8][128, H, 33] bf16."""
        nc = self.nc
        W = self.sb[wname]
        bias = self.sb[bname]
        tiles = []
        for st in range(NQT):
            ps = self.pst([128, D])
            for ih in range(2):
                nc.tensor.matmul(ps[:],
                                 _opt(xT[:, ih, st * 128:(st + 1) * 128]),
                                 _opt(W[:, ih, :]),
                                 start=(ih == 0), stop=(ih == 1))
            v = pool.tile([128, H, DK + 1], BF16, tag=f"{tag}{st}")
            nc.vector.tensor_tensor(
                out=v[:, :, 0:DK],
                in0=ps[:].rearrange("p (h d) -> p h d", h=H),
                in1=bias[:].rearrange("p (h d) -> p h d", h=H), op=OP.add)
            nc.vector.memset(_opt(v[:, :, DK:DK + 1]), 1.0)
            tiles.append(v)
        return tiles

    # ---------------------------------------------- decay (shared L1/2/3)
    def decay_av(self, lay, qt, hg, G, t_srcs, V, at, wts_pool, oq):
        """From scaled u (G['u']) -> eff -> t -> w -> wt -> o -> at slice.

        G['u']: [128, HG*(B+nm)] bf16, band cols then per-head coarse cols.
        t_srcs: (band_ap(i,h), coarse_ap(i,h)) callables for the score
        factor multiplying eff.
        """
        nc, sb = self.nc, self.sb
        Kt = (qt + 1) * 128
        B = BAND
        C = Kt - B
        nm = C // CF
        u = G["u"]
        # ln -> dg -> eff, all served by natural_log_exp_and_others; the
        # +1e-30 ln bias floors u so ln never emits -inf
        L = G["L"]
        nc.scalar.activation(out=L[:], in_=u[:], func=AF.Ln,
                             bias=sb["eps30"][:])
        nc.scalar.activation(out=L[:], in_=L[:], func=AF.Exp, scale=0.5)
        nc.scalar.activation(out=L[:], in_=L[:], func=AF.Exp, scale=-1.0)
        # t = eff * score-factor: band on vector (2x), coarse on pool
        t = G["t"]
        band_src, coarse_src = t_srcs
        for i in range(HG):
            h = hg * HG + i
            nc.vector.tensor_tensor(
                out=_opt(t[:, i, C:Kt]),
                in0=_opt(L[:, i * B:(i + 1) * B]),
                in1=band_src(i, h), op=OP.mult)
            if nm:
                nc.gpsimd.tensor_tensor(
                    out=_opt(t[:, i, 0:C]),
                    in0=_ap(L, HG * B + i * nm, [[1, nm], [0, CF]]),
                    in1=coarse_src(i, h), op=OP.mult)
        w = G["w"]
        nc.scalar.activation(out=w[:], in_=t[:], func=AF.Exp)
        # wt via one batched DMA xbar transpose per head-group
        o = oq.tile([128, HG, DK + 1], F32, tag="o", name="o")
        nblk = qt + 1
        wt = wts_pool.tile([128, HG * nblk, 128], BF16, tag="wt", name="wt")
        nc.sync.dma_start_transpose(
            out=wt[:], in_=_opt(w[:].rearrange("p i k -> p (i k)")))
        for i in range(HG):
            h = hg * HG + i
            for kb in range(nblk):
                nc.tensor.matmul(
                    _opt(o[:, i, :]), _opt(wt[:, i * nblk + kb, :]),
                    _opt(V[kb][:, h, :]),
                    start=(kb == 0), stop=(kb == qt), skip_group_check=True)
        # normalize -> at (bf16)
        Wg = G["sml"].tile([128, HG], F32, tag="Wg", name="Wg")
        rW = G["sml"].tile([128, HG], F32, tag="rW", name="rW")
        nc.vector.tensor_scalar_max(out=Wg[:], in0=_opt(o[:, :, DK:DK + 1]),
                                    scalar1=1e-30)
        nc.vector.reciprocal(out=rW[:], in_=Wg[:])
        nc.vector.tensor_tensor(
            out=_opt(at[:, hg * HG:hg * HG + HG, :]),
            in0=_opt(o[:, :, 0:DK]), in1=_bc(rW[:], DK), op=OP.mult)

    # ---------------------------------------------- attention (layers 1/2)
    def attention(self, lay, QT, KT, V, at_pool):
        nc, sb, tc = self.nc, self.sb, self.tc
        with contextlib.ExitStack() as actx:
            zq = actx.enter_context(
                tc.tile_pool(name=f"zq{lay}", bufs=2, space="PSUM"))
            oq = actx.enter_context(
                tc.tile_pool(name=f"oq{lay}", bufs=2, space="PSUM"))
            big = actx.enter_context(tc.tile_pool(name=f"big{lay}", bufs=2))
            mid = actx.enter_context(tc.tile_pool(name=f"mid{lay}", bufs=2))
            sml = actx.enter_context(tc.tile_pool(name=f"sml{lay}", bufs=3))
            wts = actx.enter_context(tc.tile_pool(name=f"wts{lay}", bufs=3))

            attn = []
            for qt in range(NQT):
                Kt = (qt + 1) * 128
                B = BAND
                C = Kt - B
                nm = C // CF
                at = at_pool.tile([128, H, DK], BF16, tag=f"at{lay}_{qt}")
                for hg in range(NG):
                    G = {"sml": sml}
                    # zsC: off-diag cols [0, C); zsB: diag block (the band)
                    zsC = big.tile([128, HG, C + 2] if C else [128, 2, 2],
                                   F16, tag="zsC", name="zsC")
                    zsB = mid.tile([128, HG * B], F16, tag="zsB", name="zsB")
                    for i in range(HG):
                        h = hg * HG + i
                        z = zq.tile([128, Kt], F32, tag="z", name="z")
                        lhs = self.hslice(QT, h,
                                          slice(qt * 128, qt * 128 + 128))
                        tp = ((h % 4) * DK, 0)
                        for ci in range((Kt + 511) // 512):
                            kc = ci * 512
                            cl = min(512, Kt - kc)
                            nc.tensor.matmul(
                                _opt(z[:, kc:kc + cl]), lhs,
                                self.hslice(KT, h, slice(kc, kc + cl)),
                                start=True, stop=True, tile_position=tp,
                                skip_group_check=True)
                        if C:
                            if i % 2 == 0:
                                nc.vector.tensor_scalar_mul(
                                    out=_opt(zsC[:, i, 0:C]),
                                    in0=_opt(z[:, 0:C]), scalar1=ISQ)
                            else:
                                nc.scalar.mul(
                                    out=_opt(zsC[:, i, 0:C]),
                                    in_=_opt(z[:, 0:C]), mul=ISQ)
                        nc.vector.scalar_tensor_tensor(
                            out=_opt(zsB[:, i * B:(i + 1) * B]),
                            in0=_opt(z[:, C:Kt]), scalar=ISQ,
                            in1=sb["M0s"][:], op0=OP.mult, op1=OP.add)
                    # e
                    eB = mid.tile([128, HG * B], BF16, tag="eB", name="eB")
                    nc.scalar.activation(out=eB[:], in_=zsB[:], func=AF.Exp)
                    if C:
                        eC = big.tile([128, HG, C + 2], BF16, tag="eC",
                                      name="eC")
                        nc.scalar.activation(
                            out=_opt(eC[:, :, 0:C]), in_=_opt(zsC[:, :, 0:C]),
                            func=AF.Exp)
                        nc.vector.memset(
                            _ap(eC, C, [[C + 2, HG], [1, 2]]), 0.0)
                    # band: segmented reversed scan over eB directly
                    Sb = mid.tile([128, HG * B + 1], BF16, tag="Sb",
                                  name="Sb")
                    nc.vector.memset(_opt(Sb[:, HG * B:HG * B + 1]), 0.0)
                    nc.vector.tensor_tensor_scan(
                        out=_rev(Sb[:, 0:HG * B]),
                        data0=_rev(sb["SEGB1"][:, 0:HG * B]),
                        data1=_rev(eB[:]),
                        initial=0.0, op0=OP.mult, op1=OP.add)
                    bm = sml.tile([128, HG], F32, tag="bm", name="bm")
                    nc.vector.tensor_copy(out=bm[:],
                                          in_=_ap(Sb, 0, [[B, HG]]))
                    # coarse: shifted pair-sums (vector 2x) + strided pool
                    if nm:
                        eCf = eC[:].rearrange("p i c -> p (i c)")
                        CW = C + 2
                        s2 = big.tile([128, HG * CW], BF16, tag="s2",
                                      name="s2")
                        nc.vector.tensor_tensor(
                            out=_opt(s2[:, 0:HG * CW - 1]),
                            in0=_opt(eCf[:, 0:HG * CW - 1]),
                            in1=_opt(eCf[:, 1:HG * CW]), op=OP.add)
                        b4 = mid.tile([128, HG, C // 4], BF16, tag="b4",
                                      name="b4")
                        nc.gpsimd.tensor_tensor(
                            out=b4[:], in0=_ap(s2, 0, [[CW, HG], [4, C // 4]]),
                            in1=_ap(s2, 2, [[CW, HG], [4, C // 4]]),
                            op=OP.add)
                        b8 = mid.tile([128, HG, C // 8], BF16, tag="b8",
                                      name="b8")
                        nc.gpsimd.tensor_tensor(
                            out=b8[:],
                            in0=_ap(b4, 0, [[C // 4, HG], [2, C // 8]]),
                            in1=_ap(b4, 1, [[C // 4, HG], [2, C // 8]]),
                            op=OP.add)
                        bx = mid.tile([128, HG, nm + 1], F32, tag="bx",
                                      name="bx")
                        nc.gpsimd.tensor_tensor(
                            out=_opt(bx[:, :, 0:nm]),
                            in0=_ap(b8, 0, [[C // 8, HG], [2, nm]]),
                            in1=_ap(b8, 1, [[C // 8, HG], [2, nm]]),
                            op=OP.add)
                        nc.vector.tensor_copy(
                            out=_ap(bx, nm, [[nm + 1, HG], [1, 1]]),
                            in_=_bc(bm[:], 1))
                        SBi = mid.tile([128, HG * (nm + 1)], F32, tag="SBi",
                                       name="SBi")
                        nc.vector.tensor_tensor_scan(
                            out=_rev(SBi[:]),
                            data0=_rev(sb[f"SEGC{qt}"][:]),
                            data1=_rev(_opt(bx[:].rearrange(
                                "p i m -> p (i m)"))),
                            initial=0.0, op0=OP.mult, op1=OP.add)
                        E = sml.tile([128, HG], F32, tag="E", name="E")
                        nc.vector.tensor_copy(out=E[:],
                                              in_=_ap(SBi, 0, [[nm + 1, HG]]))
                    else:
                        E = bm
                    # rEg = gamma^2 / E
                    rEg = sml.tile([128, HG], F32, tag="rEg", name="rEg")
                    nc.vector.reciprocal(out=rEg[:], in_=E[:])
                    nc.vector.tensor_tensor(
                        out=rEg[:], in0=rEg[:],
                        in1=_opt(sb[f"gam2_{lay}"][:, hg * HG:hg * HG + HG]),
                        op=OP.mult)
                    # u = S * pos * rEg (band + coarse super-tile)
                    u = mid.tile([128, HG * (B + nm)], BF16, tag="u",
                                 name="u")
                    nc.vector.tensor_tensor(
                        out=_opt(u[:, 0:HG * B]), in0=_opt(Sb[:, 1:HG * B + 1]),
                        in1=sb["POSB41"][:], op=OP.mult)
                    if nm:
                        nc.vector.tensor_tensor(
                            out=_ap(u, HG * B, [[nm, HG], [1, nm]]),
                            in0=_ap(SBi, 1, [[nm + 1, HG], [1, nm]]),
                            in1=_ap(sb["POSCR"], 64 - 8 * qt,
                                    [[0, HG], [1, nm]]),
                            op=OP.mult)
                    for i in range(HG):
                        nc.vector.tensor_scalar_mul(
                            out=_opt(u[:, i * B:(i + 1) * B]),
                            in0=_opt(u[:, i * B:(i + 1) * B]),
                            scalar1=rEg[:, i:i + 1])
                        if nm:
                            nc.vector.tensor_scalar_mul(
                                out=_opt(u[:, HG * B + i * nm:
                                           HG * B + (i + 1) * nm]),
                                in0=_opt(u[:, HG * B + i * nm:
                                           HG * B + (i + 1) * nm]),
                                scalar1=rEg[:, i:i + 1])
                    G["u"] = u
                    G["L"] = mid.tile([128, HG * (B + nm)], F16, tag="L",
                                      name="L")
                    G["t"] = big.tile([128, HG, Kt], F16, tag="t", name="t")
                    G["w"] = big.tile([128, HG, Kt], BF16, tag="w", name="w")

                    def band_src(i, h, zsB=zsB, B=B):
                        return _opt(zsB[:, i * B:(i + 1) * B])

                    def coarse_src(i, h, zsC=zsC, C=C):
                        return _opt(zsC[:, i, 0:C])
                    self.decay_av(lay, qt, hg, G, (band_src, coarse_src),
                                  V, at, wts, oq)
                attn.append(at)
            return attn

    # ---------------------------------------------- layer-3 attention
    def attention3(self, V, c3bc, ncum, cum3pad, at_pool):
        nc, sb, tc = self.nc, self.sb, self.tc
        with contextlib.ExitStack() as actx:
            oq = actx.enter_context(
                tc.tile_pool(name="oq3", bufs=2, space="PSUM"))
            big = actx.enter_context(tc.tile_pool(name="big3", bufs=2))
            mid = actx.enter_context(tc.tile_pool(name="mid3", bufs=2))
            sml = actx.enter_context(tc.tile_pool(name="sml3", bufs=3))
            wts = actx.enter_context(tc.tile_pool(name="wts3", bufs=3))

            attn = []
            for qt in range(NQT):
                Kt = (qt + 1) * 128
                B = BAND
                C = Kt - B
                nm = C // CF
                at = at_pool.tile([128, H, DK], BF16, tag=f"at3_{qt}")
                # E3 column (strict-causal prefix mass at row q)
                e3ps = self.pst([128, 8])
                nc.tensor.transpose(
                    e3ps[:], _opt(cum3pad[:, qt * 128:qt * 128 + 128]),
                    _opt(sb["IDF"][0:8, 0:8]))
                E3 = sml.tile([128, H], F32, tag="E3", name="E3")
                nc.vector.tensor_scalar_max(out=E3[:], in0=e3ps[:],
                                            scalar1=1e-30)
                rE3g = sml.tile([128, H], F32, tag="rE3g", name="rE3g")
                nc.vector.reciprocal(out=rE3g[:], in_=E3[:])
                nc.vector.tensor_tensor(out=rE3g[:], in0=rE3g[:],
                                        in1=sb["gam2_3"][:], op=OP.mult)
                for hg in range(NG):
                    G = {"sml": sml}
                    u = mid.tile([128, HG * (B + nm)], BF16, tag="u3",
                                 name="u3")
                    for i in range(HG):
                        h = hg * HG + i
                        # u = max(E3 + ncum, 0) * pos * rE3g, all on vector
                        nc.vector.scalar_tensor_tensor(
                            out=_opt(u[:, i * B:(i + 1) * B]),
                            in0=_opt(ncum[:, h, C:Kt]),
                            scalar=_opt(E3[:, h:h + 1]),
                            in1=_opt(sb["POSB41"][:, 0:B]),
                            op0=OP.add, op1=OP.mult)
                        nc.vector.tensor_scalar_mul(
                            out=_opt(u[:, i * B:(i + 1) * B]),
                            in0=_opt(u[:, i * B:(i + 1) * B]),
                            scalar1=rE3g[:, h:h + 1])
                        if nm:
                            nc.vector.scalar_tensor_tensor(
                                out=_ap(u, HG * B + i * nm, [[1, nm]]),
                                in0=_ap(ncum, (CF - 1) + h * SEQ,
                                        [[CF, nm]]),
                                scalar=_opt(E3[:, h:h + 1]),
                                in1=_ap(sb["POSCR"], 64 - 8 * qt, [[1, nm]]),
                                op0=OP.add, op1=OP.mult)
                            nc.vector.tensor_scalar_mul(
                                out=_ap(u, HG * B + i * nm, [[1, nm]]),
                                in0=_ap(u, HG * B + i * nm, [[1, nm]]),
                                scalar1=rE3g[:, h:h + 1])
                    G["u"] = u
                    G["L"] = mid.tile([128, HG * (B + nm)], F16, tag="L3",
                                      name="L3")
                    G["t"] = big.tile([128, HG, Kt], F16, tag="t3", name="t3")
                    G["w"] = big.tile([128, HG, Kt], BF16, tag="w3",
                                      name="w3")
                    # c3 band (= diag block) with the strict mask
                    ccomp = mid.tile([128, HG, B], F16, tag="ccmp",
                                     name="ccmp")
                    for i in range(HG):
                        h = hg * HG + i
                        nc.vector.tensor_tensor(
                            out=_opt(ccomp[:, i, :]),
                            in0=_opt(c3bc[:, h, Kt - 128:Kt]),
                            in1=sb["M3"][:], op=OP.add)

                    def band_src(i, h, ccomp=ccomp):
                        return _opt(ccomp[:, i, :])

                    def coarse_src(i, h, c3bc=c3bc, C=C):
                        return _opt(c3bc[:, h, 0:C])
                    self.decay_av(3, qt, hg, G, (band_src, coarse_src),
                                  V, at, wts, oq)
                attn.append(at)
            return attn

    # ------------------------------------------------ out proj + LN
    def out_ln(self, lay, attn, res_tiles, hpool, spool, dram_out=None):
        nc, sb = self.nc, self.sb
        attnT = hpool.tile([128, 2, SEQ], BF16, tag=f"attnT{lay}")
        for st in range(NQT):
            nc.sync.dma_start_transpose(
                out=attnT[:, :, st * 128:(st + 1) * 128],
                in_=_opt(attn[st][:].rearrange("p h d -> p (h d)")))
        W = sb[f"WT_o{lay}"]
        out_tiles = []
        for st in range(NQT):
            ps = self.pst([128, D])
            for ih in range(2):
                nc.tensor.matmul(ps[:],
                                 _opt(attnT[:, ih, st * 128:(st + 1) * 128]),
                                 _opt(W[:, ih, :]), start=(ih == 0),
                                 stop=(ih == 1))
            res = res_tiles[st] if isinstance(res_tiles, list) else res_tiles
            x = spool.tile([128, D], F32, tag="lnx")
            nc.vector.tensor_tensor(out=x[:], in0=ps[:], in1=res[:], op=OP.add)
            stats = spool.tile([128, 6], F32, tag="bnst")
            mv = spool.tile([128, 2], F32, tag="bnmv")
            nc.vector.bn_stats(out=stats[:], in_=x[:])
            nc.vector.bn_aggr(out=mv[:], in_=stats[:])
            # 1/std = exp(-0.5*ln(var+eps)) -- stays in the ln/exp table set
            lv = spool.tile([128, 1], F32, tag="lv")
            nc.scalar.activation(out=lv[:], in_=_opt(mv[:, 1:2]), func=AF.Ln,
                                 bias=sb["eps"][:], scale=1.0)
            rstd = spool.tile([128, 1], F32, tag="rstd")
            nc.scalar.activation(out=rstd[:], in_=lv[:], func=AF.Exp,
                                 scale=-0.5)
            xn = spool.tile([128, D], F32, tag="lnxn")
            nc.vector.tensor_scalar(
                out=xn[:], in0=x[:], scalar1=_opt(mv[:, 0:1]), scalar2=rstd[:],
                op0=OP.subtract, op1=OP.mult)
            ho = hpool.tile([128, D], BF16, tag=f"h{lay}_{st}")
            nc.vector.tensor_tensor(out=ho[:], in0=xn[:],
                                    in1=sb[f"lng{lay}_r"][:], op=OP.mult)
            nc.vector.tensor_tensor(out=ho[:], in0=ho[:],
                                    in1=sb[f"lnb{lay}_r"][:], op=OP.add)
            if dram_out is not None:
                o32 = spool.tile([128, D], F32, tag="ho32", name="ho32")
                nc.vector.tensor_copy(out=o32[:], in_=ho[:])
                nc.sync.dma_start(out=dram_out[st * 128:(st + 1) * 128, :],
                                  in_=o32[:])
            out_tiles.append(ho)
        return out_tiles

    def transpose_bf(self, tiles, pool, tag):
        """bf16 natural tiles [8][128, D] -> [128, 2, SEQ] via DMA xbar."""
        nc = self.nc
        xT = pool.tile([128, 2, SEQ], BF16, tag=tag)
        for st in range(NQT):
            nc.sync.dma_start_transpose(
                out=xT[:, :, st * 128:(st + 1) * 128], in_=tiles[st][:])
        return xT

    # ------------------------------------------------ layer-3 prologue
    def l3_rows(self, h1T, mpool, lpool, dram_rows):
        nc, sb = self.nc, self.sb
        KT3 = self.proj_T(h1T, "WT_k3", "bk3_c", lpool, tag="KT3")
        c3 = mpool.tile([8, SEQ], F32, tag="c3")
        for scc in range(2):
            ps = self.pst([8, 512])
            for ih in range(2):
                nc.tensor.matmul(ps[:], _opt(sb["q3blk"][:, ih, :]),
                                 _opt(KT3[:, ih, scc * 512:(scc + 1) * 512]),
                                 start=(ih == 0), stop=(ih == 1))
            nc.vector.tensor_scalar_mul(
                out=_opt(c3[:, scc * 512:(scc + 1) * 512]), in0=ps[:],
                scalar1=ISQ)
        e3 = mpool.tile([8, SEQ], F32, tag="e3")
        nc.scalar.activation(out=e3[:], in_=c3[:], func=AF.Exp)
        cum3pad = lpool.tile([8, SEQ + 128], F32, tag="cum3pad")
        nc.vector.memset(_opt(cum3pad[:, 0:1]), 0.0)
        nc.vector.tensor_tensor_scan(
            out=_opt(cum3pad[:, 1:SEQ + 1]), data0=e3[:], data1=e3[:],
            initial=0.0, op0=OP.add, op1=OP.bypass)
        nc.vector.memset(_opt(cum3pad[:, SEQ + 1:]), 0.0)
        # rows -> DRAM -> partition-broadcast back
        c3b = mpool.tile([8, SEQ], F16, tag="c3b")
        nc.vector.tensor_copy(out=c3b[:], in_=c3[:])
        ncm = mpool.tile([8, SEQ], F32, tag="ncm")
        nc.vector.tensor_scalar_mul(out=ncm[:], in0=_opt(cum3pad[:, 1:SEQ + 1]),
                                    scalar1=-1.0)
        d_c3, d_nc = dram_rows
        nc.sync.dma_start(out=d_c3[:], in_=c3b[:])
        nc.sync.dma_start(out=d_nc[:], in_=ncm[:])
        c3bc = lpool.tile([128, H, SEQ], F16, tag="c3bc")
        dap = d_c3[:]
        nc.sync.dma_start(out=c3bc[:], in_=bass.AP(
            tensor=dap.tensor, offset=0,
            ap=[[0, 128], [SEQ, H], [1, SEQ]]))
        ncum = lpool.tile([128, H, SEQ], F32, tag="ncum")
        dap = d_nc[:]
        nc.sync.dma_start(out=ncum[:], in_=bass.AP(
            tensor=dap.tensor, offset=0,
            ap=[[0, 128], [SEQ, H], [1, SEQ]]))
        return c3bc, ncum, cum3pad

    # ------------------------------------------------ final mixture
    def final(self, hh, al_dram, out_dram, spool, tpool):
        nc, sb = self.nc, self.sb
        hhT = self.transpose_bf(hh, tpool, tag="hhT")
        for st in range(NQT):
            al = spool.tile([128, H], F32, tag="al", name="al")
            nc.sync.dma_start(out=al[:],
                              in_=al_dram[st * 128:(st + 1) * 128, :])
            acc = spool.tile([128, D], F32, tag="facc", name="facc")
            for h in range(H):
                vps = self.pst([128, D])
                o = (h % 4) * DK
                nc.tensor.matmul(
                    vps[:],
                    self.hslice(hhT, h, slice(st * 128, st * 128 + 128)),
                    _opt(sb["WlvT"][o:o + DK, :]),
                    start=True, stop=False,
                    tile_position=(o, 0),
                    skip_group_check=True)
                nc.tensor.matmul(
                    vps[:], _opt(sb["ONES4"][o:o + 1, :]),
                    _opt(sb["blv_r"][o:o + 1, :]),
                    start=False, stop=True, tile_position=(o, 0),
                    skip_group_check=True)
                vsb = spool.tile([128, D], BF16, tag="vsb", name="vsb")
                nc.scalar.activation(out=vsb[:], in_=vps[:], func=AF.Sigmoid)
                if h == 0:
                    nc.vector.tensor_scalar_mul(
                        out=acc[:], in0=vsb[:], scalar1=al[:, 0:1])
                else:
                    nc.vector.scalar_tensor_tensor(
                        out=acc[:], in0=vsb[:], scalar=al[:, h:h + 1],
                        in1=acc[:], op0=OP.mult, op1=OP.add)
            nc.sync.dma_start(out=out_dram[st * 128:(st + 1) * 128, :],
                              in_=acc[:])


class _StageDoneExc(Exception):
    pass


_StageDone = _StageDoneExc()


def _patched_act_tables(nc):
    import types
    from concourse.hw_specs import get_activation_tables
    import concourse.bass_primitives_rust as _bpr

    def patched(self):
        has_act = any(isinstance(i, mybir.InstActivation)
                      for b in self.main_func.blocks
                      for i in b.instructions)
        if not has_act:
            return
        tables = []
        for name, fns in get_activation_tables(self.m.arch).items():
            if name in ("exp_and_others", "natural_log", "exp_and_friends"):
                fns = set()
            tables.append((name, fns))
        from concourse import bacc as _bacc
        _bacc._bass_rust.insert_act_table_loads(self, tables)

    nc.insert_act_table_loads = types.MethodType(patched, nc)


def build(derived, debug=False, stage=None):
    stage = stage or os.environ.get("V2_STAGE", "full")
    nc = bacc.Bacc(None, target_bir_lowering=False)
    _patched_act_tables(nc)
    dd = {}
    for name, arr in derived.items():
        if name.startswith("_"):
            continue
        dt = {np.dtype(np.float32): F32, np.dtype(bf16): BF16,
              np.dtype(f16): F16}[np.dtype(arr.dtype)]
        dd[name] = nc.dram_tensor(name, list(arr.shape), dt,
                                  kind="ExternalInput")
    for nm, shape, dt in (("xbf_q", [SEQ, D], BF16), ("xbf_s", [SEQ, D], BF16),
                          ("xr_q", [SEQ, D], F32), ("xr_s", [SEQ, D], F32),
                          ("al_nat", [SEQ, H], F32)):
        dd[nm] = nc.dram_tensor(nm, shape, dt, kind="ExternalInput")
    out = nc.dram_tensor("out", [SEQ, D], F32, kind="ExternalOutput")
    d_c3 = nc.dram_tensor("rows_c3", [8, SEQ], F16, kind="Internal")
    d_nc = nc.dram_tensor("rows_nc", [8, SEQ], F32, kind="Internal")

    def dbg(name):
        return nc.dram_tensor(name, [SEQ, D], F32,
                              kind="ExternalOutput") if debug else None

    with tile.TileContext(nc) as tc, contextlib.ExitStack() as ctx:
      try:
        kb = KB(nc, tc, ctx)
        kb.pps = ctx.enter_context(
            tc.tile_pool(name="pps", bufs=2, space="PSUM"))
        kb.load_consts(dd)
        glob = ctx.enter_context(tc.tile_pool(name="glob", bufs=1))

        def dump_at(attn, name):
            if not debug:
                return
            t = nc.dram_tensor(name, [SEQ, D], F32, kind="ExternalOutput")
            with tc.tile_pool(name=f"dbg_{name}", bufs=2) as dp:
                for st in range(NQT):
                    o32 = dp.tile([128, D], F32, tag=f"d{st}")
                    nc.vector.tensor_copy(
                        out=o32[:].rearrange("p (h d) -> p h d", h=H),
                        in_=attn[st][:])
                    nc.sync.dma_start(out=t[st * 128:(st + 1) * 128, :],
                                      in_=o32[:])

        # ---------------- layer 1 (on x_q) ----------------
        with tc.tile_pool(name="r1", bufs=1) as r1, \
                tc.tile_pool(name="r1s", bufs=2) as r1s:
            xT_q = r1.tile([128, 2, SEQ], BF16, tag="xTq")
            nc.sync.dma_start_transpose(out=xT_q[:], in_=dd["xbf_q"][:])
            xr1 = []
            for st in range(NQT):
                t = r1.tile([128, D], F32, tag=f"xr1_{st}")
                nc.sync.dma_start(out=t[:],
                                  in_=dd["xr_q"][st * 128:(st + 1) * 128, :])
                xr1.append(t)
            QT1 = kb.proj_T(xT_q, "WT_q1", "bq1_c", r1, tag="QT1")
            V1 = kb.proj_V(xT_q, "WT_v1", "bv1_r", r1, tag="V1")
            attn1 = kb.attention(1, QT1, QT1, V1, r1)
            dump_at(attn1, "dbg_attn1")
            h1 = kb.out_ln(1, attn1, xr1, r1, r1s, dram_out=dbg("dbg_h1"))
            if stage == "l1":
                for st in range(NQT):
                    o32 = r1s.tile([128, D], F32, tag="s1o")
                    nc.vector.tensor_copy(out=o32[:], in_=h1[st][:])
                    nc.sync.dma_start(out=out[st * 128:(st + 1) * 128, :],
                                      in_=o32[:])
            h1T = kb.transpose_bf(h1, glob, tag="h1T")
        # ---------------- layer 2 (on x_s) ----------------
        with tc.tile_pool(name="r2", bufs=1) as r2, \
                tc.tile_pool(name="r2s", bufs=2) as r2s:
            if stage == "l1":
                raise _StageDone
            xT_s = r2.tile([128, 2, SEQ], BF16, tag="xTs")
            nc.sync.dma_start_transpose(out=xT_s[:], in_=dd["xbf_s"][:])
            xr2 = []
            for st in range(NQT):
                t = r2.tile([128, D], F32, tag=f"xr2_{st}")
                nc.sync.dma_start(out=t[:],
                                  in_=dd["xr_s"][st * 128:(st + 1) * 128, :])
                xr2.append(t)
            QT2 = kb.proj_T(xT_s, "WT_q2", "bq2_c", r2, tag="QT2")
            V2 = kb.proj_V(xT_s, "WT_v2", "bv2_r", r2, tag="V2")
            attn2 = kb.attention(2, QT2, QT2, V2, r2)
            h2 = kb.out_ln(2, attn2, xr2, r2, r2s, dram_out=dbg("dbg_h2"))
            if stage == "l2":
                for st in range(NQT):
                    o32 = r2s.tile([128, D], F32, tag="s2o")
                    nc.vector.tensor_copy(out=o32[:], in_=h2[st][:])
                    nc.sync.dma_start(out=out[st * 128:(st + 1) * 128, :],
                                      in_=o32[:])
            h2T = glob.tile([128, 2, SEQ], BF16, tag="h2T")
            for st in range(NQT):
                nc.sync.dma_start_transpose(
                    out=h2T[:, :, st * 128:(st + 1) * 128], in_=h2[st][:])
        # ---------------- layer 3 ----------------
        if stage == "l2":
            raise _StageDone
        lpool = ctx.enter_context(tc.tile_pool(name="l3pool", bufs=1))
        with tc.tile_pool(name="l3tmp", bufs=1) as l3tmp:
            V3 = kb.proj_V(h2T, "WT_v3", "bv3_r", lpool, tag="V3")
            c3bc, ncum, cum3pad = kb.l3_rows(h1T, l3tmp, lpool,
                                             (d_c3, d_nc))
            if stage == "pro":
                for st in range(NQT):
                    o32 = l3tmp.tile([128, D], F32, tag=f"s3o{st}")
                    nc.vector.tensor_copy(
                        out=o32[:].rearrange("p (h d) -> p h d", h=H),
                        in_=V3[st][:, :, 0:DK])
                    nc.sync.dma_start(out=out[st * 128:(st + 1) * 128, :],
                                      in_=o32[:])
        if stage == "pro":
            raise _StageDone
        with tc.tile_pool(name="r3", bufs=1) as r3, \
                tc.tile_pool(name="r3s", bufs=2) as r3s:
            attn3 = kb.attention3(V3, c3bc, ncum, cum3pad, r3)
            dump_at(attn3, "dbg_attn3")
            if stage == "a3":
                for st in range(NQT):
                    o32 = r3s.tile([128, D], F32, tag="s4o")
                    nc.vector.tensor_copy(
                        out=o32[:].rearrange("p (h d) -> p h d", h=H),
                        in_=attn3[st][:])
                    nc.sync.dma_start(out=out[st * 128:(st + 1) * 128, :],
                                      in_=o32[:])
                raise _StageDone
            hh = kb.out_ln(3, attn3, kb.sb["knowr_r"], r3, r3s,
                           dram_out=dbg("dbg_hh"))
            if stage == "hh":
                for st in range(NQT):
                    o32 = r3s.tile([128, D], F32, tag="s5o")
                    nc.vector.tensor_copy(out=o32[:], in_=hh[st][:])
                    nc.sync.dma_start(out=out[st * 128:(st + 1) * 128, :],
                                      in_=o32[:])
                raise _StageDone
            kb.final(hh, dd["al_nat"], out, r3s, r3)
      except _StageDoneExc:
        pass
    nc.compile()
    return nc


_CACHE = {}


def kernel(**inputs):
    drv = host_prep(inputs)
    if "nc" not in _CACHE:
        _CACHE["nc"] = build(drv)
    nc = _CACHE["nc"]
    in_maps = per_batch_maps(inputs, drv)
    from concourse.bass_utils import run_bass_kernel_spmd
    res = run_bass_kernel_spmd(nc, in_maps, core_ids=list(range(BS)))
    out = np.stack([np.asarray(res.results[b]["out"]) for b in range(BS)],
                   axis=0)
    return out.astype(np.float32)


if __name__ == "__main__":
    print("kernel module loaded OK")



# revision 54
# speedup vs baseline: 1.3059x; 1.0317x over previous
"""Trainium2 Bass kernel for nn_DTransformer (sparse attention w/ distance decay).

Sharding: data-parallel over batch (bs=8 -> 8 cores, weights replicated).

v2 design notes:
- All PE matmuls in bf16 (fp32 is 4 cyc/row on the PE; bf16 is 1).
- Causal mask folded into the zs write (STT on the diag block) - no PE mask
  matmuls; no rank-1 bias matmuls (residuals pre-biased on host, alpha of the
  final mixture computed on host, blv added on vector).
- A@V weight transposes done by the DMA xbar (dma_start_transpose), one
  descriptor per (head, q-tile), not by the PE.
- Distance-decay eff computed at full resolution only in a 256-wide diagonal
  band; beyond that on a stride-16 coarse grid (validated: 3e-5 output err).
  Suffix masses: segmented reversed scan over a compacted band (poison-mask
  trick resets at head boundaries) chained with a tiny scan over coarse block
  sums built by pairwise adds on the (otherwise idle) GPSIMD engine.
- sqrt via exp(0.5*ln(u)): ln+exp live in one ACT table set, so the scalar
  engine never swaps tables inside the hot loop (sqrt would force 2 swaps per
  q-tile). LayerNorm's 1/std likewise uses exp(-0.5*ln(var+eps)).
- zs / t / block-sum elementwise passes run on GPSIMD (Pool) to unload the
  vector engine.
"""

import os
import sys
import contextlib

for _p in ("/opt/trn_rl_repo", "/root/.axon_site/_ro/trn_rl_repo"):
    if os.path.isdir(_p) and _p not in sys.path:
        sys.path.insert(0, _p)

import numpy as np
import ml_dtypes

import concourse.bass as bass
import concourse.mybir as mybir
import concourse.tile as tile
from concourse import bacc

F32 = mybir.dt.float32
F16 = mybir.dt.float16
BF16 = mybir.dt.bfloat16
AF = mybir.ActivationFunctionType
OP = mybir.AluOpType

D = 256
H = 8
HG = 4            # heads per group
NG = H // HG
DK = 32
SEQ = 1024
BS = 8
NQT = SEQ // 128
ISQ = float(1.0 / np.sqrt(np.float32(DK)))
MASKF = -53000.0   # fits f16; exp() underflows to exactly 0
EPS = 1e-5
BAND = 128         # full-res band width (= diag block)
CF = 16            # coarse cell width

bf16 = ml_dtypes.bfloat16
f16 = np.float16
KEEP0 = frozenset({0})


def _opt(ap):
    return ap.opt(keep_dims=KEEP0)


def _rev(ap):
    """Reverse the innermost free dim of an AP (squeeze count-1 dims)."""
    pairs = [list(x) for x in ap.ap]
    keep = [pairs[0]] + [x for x in pairs[1:] if x[1] != 1]
    assert len(keep) == 2, f"need 2D-able ap, got {ap.ap}"
    (ps, pc), (fs, fc) = keep
    return bass.AP(tensor=ap.tensor, offset=ap.offset + fs * (fc - 1),
                   ap=[[ps, pc], [-fs, fc]])


def _ap(t, offset, dims):
    """Build an AP on tile t's tensor with explicit [stride, count] dims."""
    base = t[:]
    return bass.AP(tensor=base.tensor, offset=base.offset + offset,
                   ap=[list(base.ap[0])] + [list(d) for d in dims])


def _bc(ap, n):
    pairs = [list(x) for x in ap.ap]
    return bass.AP(tensor=ap.tensor, offset=ap.offset, ap=pairs + [[0, n]])


# ---------------------------------------------------------------- host prep

def host_prep(inputs):
    g = {k: np.asarray(v) for k, v in inputs.items()}

    def f32(x):
        return np.ascontiguousarray(np.asarray(x, dtype=np.float32))

    def as_bf(x):
        return np.ascontiguousarray(np.asarray(x, np.float32).astype(bf16))

    drv = {}
    for i, names in ((1, ("q", "v", "o")), (2, ("q", "v", "o")),
                     (3, ("k", "v", "o"))):
        for n in names:
            drv[f"WT_{n}{i}"] = as_bf(g[f"W{n}{i}"].T)      # [din, dout]
    for nm in ("bq1", "bq2", "bk3"):
        drv[nm + "_c"] = f32(np.asarray(g[nm], np.float32).reshape(2, 128).T)
    for nm in ("bv1", "bv2", "bv3"):
        drv[nm + "_r"] = f32(g[nm]).reshape(1, D)
    drv["blv_r"] = f32(g["blv"]).reshape(1, D)
    for i in (1, 2, 3):
        drv[f"lng{i}_r"] = f32(g[f"lng{i}"]).reshape(1, D)
        drv[f"lnb{i}_r"] = f32(g[f"lnb{i}"]).reshape(1, D)
        gam = -np.logaddexp(0.0, f32(g[f"g{i}"]).reshape(H))
        drv[f"gam2_{i}"] = f32((gam * gam).reshape(1, H))
    know = f32(g["know"]).reshape(D)
    q3 = know @ f32(g["Wq3"]).T + f32(g["bq3"])
    q3blk = np.zeros((D, H), np.float32)
    for h in range(H):
        q3blk[h * DK:(h + 1) * DK, h] = q3[h * DK:(h + 1) * DK]
    drv["q3blk"] = as_bf(q3blk)
    drv["knowr_r"] = f32(know + f32(g["bo3"])).reshape(1, D)  # pre-biased res
    drv["WlvT"] = as_bf(np.tile(g["Wlv"].T, (4, 1)))          # [128, 256] x4

    p = np.arange(128, dtype=np.float32)[:, None]
    j1 = np.arange(128, dtype=np.float32)[None, :]
    posb1 = np.maximum(p - j1, 0.0)                  # band == diag block
    drv["POSB41"] = np.ascontiguousarray(
        np.tile(posb1, (1, HG)).astype(bf16))        # [128, 4*128]
    # descending coarse pos table: col jj -> 16*(64-jj) + p - 7.5; the
    # per-qt slice [64-8qt : 64-8qt+nm] walks cells m=0..nm-1 ascending
    jj = np.arange(72, dtype=np.float32)[None, :]
    drv["POSCR"] = np.ascontiguousarray(
        (CF * (64.0 - jj) + p - (CF - 1) / 2.0).astype(bf16))  # [128, 72]

    segb1 = np.ones((128, HG * 128), np.float32)
    segb1[:, 127::128] = 0.0
    drv["SEGB1"] = np.ascontiguousarray(segb1.astype(bf16))
    for qt in range(1, NQT):
        nm = qt * 128 // CF
        sc = np.ones((128, HG * (nm + 1)), np.float32)
        sc[:, nm::nm + 1] = 0.0
        drv[f"SEGC{qt}"] = np.ascontiguousarray(sc.astype(bf16))
    drv["ONES4"] = f32(np.ones((128, 128)))

    jj = np.arange(128)[None, :]
    drv["M0s"] = np.ascontiguousarray(
        np.where(jj <= np.arange(128)[:, None], 0.0, MASKF).astype(f16))
    drv["M3"] = np.ascontiguousarray(
        np.where(jj < np.arange(128)[:, None], 0.0, -6e4).astype(f16))
    drv["IDF"] = f32(np.eye(128))

    # host-side alpha for the final mixture (tiny: [bs, s, h])
    kk = know.reshape(H, DK) @ f32(g["Wlk"]).T + f32(g["blk"])
    kk = 1.0 / (1.0 + np.exp(-kk))
    q = np.asarray(g["q_emb"], np.float32)
    beta = np.einsum("hd,bsd->bsh", kk, q)
    beta -= beta.max(-1, keepdims=True)
    ee = np.exp(beta)
    drv["_alpha"] = (ee / ee.sum(-1, keepdims=True)).astype(np.float32)
    return drv


def per_batch_maps(inputs, drv):
    q = np.asarray(inputs["q_emb"], np.float32)
    s = np.asarray(inputs["s_emb"], np.float32)
    bo1 = np.asarray(inputs["bo1"], np.float32)
    bo2 = np.asarray(inputs["bo2"], np.float32)
    alpha = drv["_alpha"]
    base = {k: v for k, v in drv.items() if not k.startswith("_")}
    maps = []
    for b in range(BS):
        m = dict(base)
        m["xbf_q"] = np.ascontiguousarray(q[b].astype(bf16))
        m["xbf_s"] = np.ascontiguousarray(s[b].astype(bf16))
        m["xr_q"] = np.ascontiguousarray(q[b] + bo1)
        m["xr_s"] = np.ascontiguousarray(s[b] + bo2)
        m["al_nat"] = np.ascontiguousarray(alpha[b])
        maps.append(m)
    return maps


# ---------------------------------------------------------------- builder

class KB:
    def __init__(self, nc, tc, ctx):
        self.nc, self.tc, self.ctx = nc, tc, ctx

    def pst(self, shape):
        return self.pps.tile(shape, F32, tag="ps", name="ps")

    def load_consts(self, dd):
        nc = self.nc
        pool = self.ctx.enter_context(self.tc.tile_pool(name="consts", bufs=1))
        sb = {}
        for i, names in ((1, ("q", "v", "o")), (2, ("q", "v", "o")),
                         (3, ("k", "v", "o"))):
            for n in names:
                t = pool.tile([128, 2, D], BF16, tag=f"WT_{n}{i}")
                nc.sync.dma_start(
                    out=t[:],
                    in_=dd[f"WT_{n}{i}"][:].rearrange("(a p) d -> p a d", p=128))
                sb[f"WT_{n}{i}"] = t
        t = pool.tile([128, 2, H], BF16, tag="q3blk")
        nc.sync.dma_start(
            out=t[:], in_=dd["q3blk"][:].rearrange("(a p) h -> p a h", p=128))
        sb["q3blk"] = t
        for nm in list(dd.keys()):
            if nm.startswith(("POSB", "POSC", "SEGB", "SEGC", "M0s", "M3",
                              "IDF", "WlvT")) or nm.endswith("_c"):
                src = dd[nm]
                t = pool.tile(list(src.shape), src.dtype, tag=nm)
                nc.sync.dma_start(out=t[:], in_=src[:])
                sb[nm] = t
        for nm in ("bv1_r", "bv2_r", "bv3_r", "lng1_r", "lng2_r", "lng3_r",
                   "lnb1_r", "lnb2_r", "lnb3_r", "knowr_r", "gam2_1",
                   "gam2_2", "gam2_3", "blv_r"):
            src = dd[nm]
            n = src.shape[1]
            t = pool.tile([128, n], F32, tag=nm)
            nc.sync.dma_start(
                out=t[:],
                in_=bass.AP(tensor=src, offset=0, ap=[[0, 128], [1, n]]))
            sb[nm] = t
        t = pool.tile([128, 128], F32, tag="ONES4")
        nc.sync.dma_start(out=t[:], in_=dd["ONES4"][:])
        sb["ONES4"] = t
        epst = pool.tile([128, 1], F32, tag="eps")
        nc.vector.memset(epst[:], EPS)
        sb["eps"] = epst
        e30 = pool.tile([128, 1], F32, tag="eps30")
        nc.vector.memset(e30[:], 1e-30)
        sb["eps30"] = e30
        self.sb = sb
        # warm the PE transpose path (single sync wait on LDWEIGHTS)
        junk = pool.tile([128, 1], F32, tag="junk")
        wf = self.pps.tile([128, 128], F32, tag="ps", name="warmf")
        nc.tensor.transpose(wf[:], sb["IDF"][:], sb["IDF"][:])
        nc.scalar.copy(out=junk[:, 0:1], in_=wf[:, 0:1])

    def hslice(self, T, h, cols):
        return _opt(T[(h % 4) * DK:(h % 4 + 1) * DK, h // 4, cols])

    def proj_T(self, xT, wname, bname, pool, tag):
        """out[do, s] = W @ x.T + b : [128, 2, 1024] bf16."""
        nc = self.nc
        W = self.sb[wname]
        out = pool.tile([128, 2, SEQ], BF16, tag=tag)
        for dh in range(2):
            for sc in range(2):
                ps = self.pst([128, 512])
                for ih in range(2):
                    nc.tensor.matmul(
                        ps[:], _opt(W[:, ih, dh * 128:(dh + 1) * 128]),
                        _opt(xT[:, ih, sc * 512:(sc + 1) * 512]),
                        start=(ih == 0), stop=(ih == 1))
                nc.scalar.activation(
                    out=_opt(out[:, dh, sc * 512:(sc + 1) * 512]), in_=ps[:],
                    func=AF.Identity, bias=self.sb[bname][:, dh:dh + 1],
                    scale=1.0)
        return out

    def proj_V(self, xT, wname, bname, pool, tag):
        """V natural with ones column: [8][128, H, 33] bf16."""
        nc = self.nc
        W = self.sb[wname]
        bias = self.sb[bname]
        tiles = []
        for st in range(NQT):
            ps = self.pst([128, D])
            for ih in range(2):
                nc.tensor.matmul(ps[:],
                                 _opt(xT[:, ih, st * 128:(st + 1) * 128]),
                                 _opt(W[:, ih, :]),
                                 start=(ih == 0), stop=(ih == 1))
            v = pool.tile([128, H, DK + 1], BF16, tag=f"{tag}{st}")
            nc.vector.tensor_tensor(
                out=v[:, :, 0:DK],
                in0=ps[:].rearrange("p (h d) -> p h d", h=H),
                in1=bias[:].rearrange("p (h d) -> p h d", h=H), op=OP.add)
            nc.vector.memset(_opt(v[:, :, DK:DK + 1]), 1.0)
            tiles.append(v)
        return tiles

    # ---------------------------------------------- decay (shared L1/2/3)
    def decay_av(self, lay, qt, hg, G, t_srcs, V, at, wts_pool, oq):
        """From scaled u (G['u']) -> eff -> t -> w -> wt -> o -> at slice.

        G['u']: [128, HG*(B+nm)] bf16, band cols then per-head coarse cols.
        t_srcs: (band_ap(i,h), coarse_ap(i,h)) callables for the score
        factor multiplying eff.
        """
        nc, sb = self.nc, self.sb
        Kt = (qt + 1) * 128
        B = BAND
        C = Kt - B
        nm = C // CF
        u = G["u"]
        # ln -> dg -> eff, all served by natural_log_exp_and_others; the
        # +1e-30 ln bias floors u so ln never emits -inf
        L = G["L"]
        nc.scalar.activation(out=L[:], in_=u[:], func=AF.Ln,
                             bias=sb["eps30"][:])
        nc.scalar.activation(out=L[:], in_=L[:], func=AF.Exp, scale=0.5)
        nc.scalar.activation(out=L[:], in_=L[:], func=AF.Exp, scale=-1.0)
        # t = eff * score-factor: band on vector (2x), coarse on pool
        t = G["t"]
        band_src, coarse_src = t_srcs
        for i in range(HG):
            h = hg * HG + i
            nc.vector.tensor_tensor(
                out=_opt(t[:, i, C:Kt]),
                in0=_opt(L[:, i * B:(i + 1) * B]),
                in1=band_src(i, h), op=OP.mult)
            if nm:
                nc.gpsimd.tensor_tensor(
                    out=_opt(t[:, i, 0:C]),
                    in0=_ap(L, HG * B + i * nm, [[1, nm], [0, CF]]),
                    in1=coarse_src(i, h), op=OP.mult)
        w = G["w"]
        nc.scalar.activation(out=w[:], in_=t[:], func=AF.Exp)
        # wt via one batched DMA xbar transpose per head-group
        o = oq.tile([128, HG, DK + 1], F32, tag="o", name="o")
        nblk = qt + 1
        wt = wts_pool.tile([128, HG * nblk, 128], BF16, tag="wt", name="wt")
        nc.sync.dma_start_transpose(
            out=wt[:], in_=_opt(w[:].rearrange("p i k -> p (i k)")))
        for i in range(HG):
            h = hg * HG + i
            for kb in range(nblk):
                nc.tensor.matmul(
                    _opt(o[:, i, :]), _opt(wt[:, i * nblk + kb, :]),
                    _opt(V[kb][:, h, :]),
                    start=(kb == 0), stop=(kb == qt), skip_group_check=True)
        # normalize -> at (bf16)
        Wg = G["sml"].tile([128, HG], F32, tag="Wg", name="Wg")
        rW = G["sml"].tile([128, HG], F32, tag="rW", name="rW")
        nc.vector.tensor_scalar_max(out=Wg[:], in0=_opt(o[:, :, DK:DK + 1]),
                                    scalar1=1e-30)
        nc.vector.reciprocal(out=rW[:], in_=Wg[:])
        nc.vector.tensor_tensor(
            out=_opt(at[:, hg * HG:hg * HG + HG, :]),
            in0=_opt(o[:, :, 0:DK]), in1=_bc(rW[:], DK), op=OP.mult)

    # ---------------------------------------------- attention (layers 1/2)
    def attention(self, lay, QT, KT, V, at_pool):
        nc, sb, tc = self.nc, self.sb, self.tc
        with contextlib.ExitStack() as actx:
            zq = actx.enter_context(
                tc.tile_pool(name=f"zq{lay}", bufs=2, space="PSUM"))
            oq = actx.enter_context(
                tc.tile_pool(name=f"oq{lay}", bufs=2, space="PSUM"))
            big = actx.enter_context(tc.tile_pool(name=f"big{lay}", bufs=2))
            mid = actx.enter_context(tc.tile_pool(name=f"mid{lay}", bufs=2))
            sml = actx.enter_context(tc.tile_pool(name=f"sml{lay}", bufs=3))
            wts = actx.enter_context(tc.tile_pool(name=f"wts{lay}", bufs=3))

            attn = []
            for qt in range(NQT):
                Kt = (qt + 1) * 128
                B = BAND
                C = Kt - B
                nm = C // CF
                at = at_pool.tile([128, H, DK], BF16, tag=f"at{lay}_{qt}")
                for hg in range(NG):
                    G = {"sml": sml}
                    # zsC: off-diag cols [0, C); zsB: diag block (the band)
                    zsC = big.tile([128, HG, C + 2] if C else [128, 2, 2],
                                   F16, tag="zsC", name="zsC")
                    zsB = mid.tile([128, HG * B], F16, tag="zsB", name="zsB")
                    for i in range(HG):
                        h = hg * HG + i
                        z = zq.tile([128, Kt], F32, tag="z", name="z")
                        lhs = self.hslice(QT, h,
                                          slice(qt * 128, qt * 128 + 128))
                        tp = ((h % 4) * DK, 0)
                        for ci in range((Kt + 511) // 512):
                            kc = ci * 512
                            cl = min(512, Kt - kc)
                            nc.tensor.matmul(
                                _opt(z[:, kc:kc + cl]), lhs,
                                self.hslice(KT, h, slice(kc, kc + cl)),
                                start=True, stop=True, tile_position=tp,
                                skip_group_check=True)
                        if C:
                            if i % 2 == 0:
                                nc.vector.tensor_scalar_mul(
                                    out=_opt(zsC[:, i, 0:C]),
                                    in0=_opt(z[:, 0:C]), scalar1=ISQ)
                            else:
                                nc.scalar.mul(
                                    out=_opt(zsC[:, i, 0:C]),
                                    in_=_opt(z[:, 0:C]), mul=ISQ)
                        nc.vector.scalar_tensor_tensor(
                            out=_opt(zsB[:, i * B:(i + 1) * B]),
                            in0=_opt(z[:, C:Kt]), scalar=ISQ,
                            in1=sb["M0s"][:], op0=OP.mult, op1=OP.add)
                    # e
                    eB = mid.tile([128, HG * B], BF16, tag="eB", name="eB")
                    nc.scalar.activation(out=eB[:], in_=zsB[:], func=AF.Exp)
                    if C:
                        eC = big.tile([128, HG, C + 2], BF16, tag="eC",
                                      name="eC")
                        nc.scalar.activation(
                            out=_opt(eC[:, :, 0:C]), in_=_opt(zsC[:, :, 0:C]),
                            func=AF.Exp)
                        nc.vector.memset(
                            _ap(eC, C, [[C + 2, HG], [1, 2]]), 0.0)
                    # band: segmented reversed scan over eB directly
                    Sb = mid.tile([128, HG * B + 1], BF16, tag="Sb",
                                  name="Sb")
                    nc.vector.memset(_opt(Sb[:, HG * B:HG * B + 1]), 0.0)
                    nc.vector.tensor_tensor_scan(
                        out=_rev(Sb[:, 0:HG * B]),
                        data0=_rev(sb["SEGB1"][:, 0:HG * B]),
                        data1=_rev(eB[:]),
                        initial=0.0, op0=OP.mult, op1=OP.add)
                    bm = sml.tile([128, HG], F32, tag="bm", name="bm")
                    nc.vector.tensor_copy(out=bm[:],
                                          in_=_ap(Sb, 0, [[B, HG]]))
                    # coarse: shifted pair-sums (vector 2x) + strided pool
                    if nm:
                        eCf = eC[:].rearrange("p i c -> p (i c)")
                        CW = C + 2
                        s2 = big.tile([128, HG * CW], BF16, tag="s2",
                                      name="s2")
                        nc.vector.tensor_tensor(
                            out=_opt(s2[:, 0:HG * CW - 1]),
                            in0=_opt(eCf[:, 0:HG * CW - 1]),
                            in1=_opt(eCf[:, 1:HG * CW]), op=OP.add)
                        b4 = mid.tile([128, HG, C // 4], BF16, tag="b4",
                                      name="b4")
                        nc.gpsimd.tensor_tensor(
                            out=b4[:], in0=_ap(s2, 0, [[CW, HG], [4, C // 4]]),
                            in1=_ap(s2, 2, [[CW, HG], [4, C // 4]]),
                            op=OP.add)
                        b8 = mid.tile([128, HG, C // 8], BF16, tag="b8",
                                      name="b8")
                        nc.gpsimd.tensor_tensor(
                            out=b8[:],
                            in0=_ap(b4, 0, [[C // 4, HG], [2, C // 8]]),
                            in1=_ap(b4, 1, [[C // 4, HG], [2, C // 8]]),
                            op=OP.add)
                        bx = mid.tile([128, HG, nm + 1], F32, tag="bx",
                                      name="bx")
                        nc.gpsimd.tensor_tensor(
                            out=_opt(bx[:, :, 0:nm]),
                            in0=_ap(b8, 0, [[C // 8, HG], [2, nm]]),
                            in1=_ap(b8, 1, [[C // 8, HG], [2, nm]]),
                            op=OP.add)
                        nc.vector.tensor_copy(
                            out=_ap(bx, nm, [[nm + 1, HG], [1, 1]]),
                            in_=_bc(bm[:], 1))
                        SBi = mid.tile([128, HG * (nm + 1)], F32, tag="SBi",
                                       name="SBi")
                        nc.vector.tensor_tensor_scan(
                            out=_rev(SBi[:]),
                            data0=_rev(sb[f"SEGC{qt}"][:]),
                            data1=_rev(_opt(bx[:].rearrange(
                                "p i m -> p (i m)"))),
                            initial=0.0, op0=OP.mult, op1=OP.add)
                        E = sml.tile([128, HG], F32, tag="E", name="E")
                        nc.vector.tensor_copy(out=E[:],
                                              in_=_ap(SBi, 0, [[nm + 1, HG]]))
                    else:
                        E = bm
                    # rEg = gamma^2 / E
                    rEg = sml.tile([128, HG], F32, tag="rEg", name="rEg")
                    nc.vector.reciprocal(out=rEg[:], in_=E[:])
                    nc.vector.tensor_tensor(
                        out=rEg[:], in0=rEg[:],
                        in1=_opt(sb[f"gam2_{lay}"][:, hg * HG:hg * HG + HG]),
                        op=OP.mult)
                    # u = S * pos * rEg (band + coarse super-tile)
                    u = mid.tile([128, HG * (B + nm)], BF16, tag="u",
                                 name="u")
                    nc.vector.tensor_tensor(
                        out=_opt(u[:, 0:HG * B]), in0=_opt(Sb[:, 1:HG * B + 1]),
                        in1=sb["POSB41"][:], op=OP.mult)
                    if nm:
                        nc.vector.tensor_tensor(
                            out=_ap(u, HG * B, [[nm, HG], [1, nm]]),
                            in0=_ap(SBi, 1, [[nm + 1, HG], [1, nm]]),
                            in1=_ap(sb["POSCR"], 64 - 8 * qt,
                                    [[0, HG], [1, nm]]),
                            op=OP.mult)
                    for i in range(HG):
                        nc.vector.tensor_scalar_mul(
                            out=_opt(u[:, i * B:(i + 1) * B]),
                            in0=_opt(u[:, i * B:(i + 1) * B]),
                            scalar1=rEg[:, i:i + 1])
                        if nm:
                            nc.vector.tensor_scalar_mul(
                                out=_opt(u[:, HG * B + i * nm:
                                           HG * B + (i + 1) * nm]),
                                in0=_opt(u[:, HG * B + i * nm:
                                           HG * B + (i + 1) * nm]),
                                scalar1=rEg[:, i:i + 1])
                    G["u"] = u
                    G["L"] = mid.tile([128, HG * (B + nm)], F16, tag="L",
                                      name="L")
                    G["t"] = big.tile([128, HG, Kt], F16, tag="t", name="t")
                    G["w"] = big.tile([128, HG, Kt], BF16, tag="w", name="w")

                    def band_src(i, h, zsB=zsB, B=B):
                        return _opt(zsB[:, i * B:(i + 1) * B])

                    def coarse_src(i, h, zsC=zsC, C=C):
                        return _opt(zsC[:, i, 0:C])
                    self.decay_av(lay, qt, hg, G, (band_src, coarse_src),
                                  V, at, wts, oq)
                attn.append(at)
            return attn

    # ---------------------------------------------- layer-3 attention
    def attention3(self, V, c3bc, ncum, cum3pad, at_pool):
        nc, sb, tc = self.nc, self.sb, self.tc
        with contextlib.ExitStack() as actx:
            oq = actx.enter_context(
                tc.tile_pool(name="oq3", bufs=2, space="PSUM"))
            big = actx.enter_context(tc.tile_pool(name="big3", bufs=2))
            mid = actx.enter_context(tc.tile_pool(name="mid3", bufs=2))
            sml = actx.enter_context(tc.tile_pool(name="sml3", bufs=3))
            wts = actx.enter_context(tc.tile_pool(name="wts3", bufs=3))

            attn = []
            for qt in range(NQT):
                Kt = (qt + 1) * 128
                B = BAND
                C = Kt - B
                nm = C // CF
                at = at_pool.tile([128, H, DK], BF16, tag=f"at3_{qt}")
                # E3 column (strict-causal prefix mass at row q)
                e3ps = self.pst([128, 8])
                nc.tensor.transpose(
                    e3ps[:], _opt(cum3pad[:, qt * 128:qt * 128 + 128]),
                    _opt(sb["IDF"][0:8, 0:8]))
                E3 = sml.tile([128, H], F32, tag="E3", name="E3")
                nc.vector.tensor_scalar_max(out=E3[:], in0=e3ps[:],
                                            scalar1=1e-30)
                rE3g = sml.tile([128, H], F32, tag="rE3g", name="rE3g")
                nc.vector.reciprocal(out=rE3g[:], in_=E3[:])
                nc.vector.tensor_tensor(out=rE3g[:], in0=rE3g[:],
                                        in1=sb["gam2_3"][:], op=OP.mult)
                for hg in range(NG):
                    G = {"sml": sml}
                    u = mid.tile([128, HG * (B + nm)], BF16, tag="u3",
                                 name="u3")
                    for i in range(HG):
                        h = hg * HG + i
                        # u = max(E3 + ncum, 0) * pos * rE3g, all on vector
                        nc.vector.scalar_tensor_tensor(
                            out=_opt(u[:, i * B:(i + 1) * B]),
                            in0=_opt(ncum[:, h, C:Kt]),
                            scalar=_opt(E3[:, h:h + 1]),
                            in1=_opt(sb["POSB41"][:, 0:B]),
                            op0=OP.add, op1=OP.mult)
                        nc.vector.tensor_scalar_mul(
                            out=_opt(u[:, i * B:(i + 1) * B]),
                            in0=_opt(u[:, i * B:(i + 1) * B]),
                            scalar1=rE3g[:, h:h + 1])
                        if nm:
                            nc.vector.scalar_tensor_tensor(
                                out=_ap(u, HG * B + i * nm, [[1, nm]]),
                                in0=_ap(ncum, (CF - 1) + h * SEQ,
                                        [[CF, nm]]),
                                scalar=_opt(E3[:, h:h + 1]),
                                in1=_ap(sb["POSCR"], 64 - 8 * qt, [[1, nm]]),
                                op0=OP.add, op1=OP.mult)
                            nc.vector.tensor_scalar_mul(
                                out=_ap(u, HG * B + i * nm, [[1, nm]]),
                                in0=_ap(u, HG * B + i * nm, [[1, nm]]),
                                scalar1=rE3g[:, h:h + 1])
                    G["u"] = u
                    G["L"] = mid.tile([128, HG * (B + nm)], F16, tag="L3",
                                      name="L3")
                    G["t"] = big.tile([128, HG, Kt], F16, tag="t3", name="t3")
                    G["w"] = big.tile([128, HG, Kt], BF16, tag="w3",
                                      name="w3")
                    # c3 band (= diag block) with the strict mask
                    ccomp = mid.tile([128, HG, B], F16, tag="ccmp",
                                     name="ccmp")
                    for i in range(HG):
                        h = hg * HG + i
                        nc.vector.tensor_tensor(
                            out=_opt(ccomp[:, i, :]),
                            in0=_opt(c3bc[:, h, Kt - 128:Kt]),
                            in1=sb["M3"][:], op=OP.add)

                    def band_src(i, h, ccomp=ccomp):
                        return _opt(ccomp[:, i, :])

                    def coarse_src(i, h, c3bc=c3bc, C=C):
                        return _opt(c3bc[:, h, 0:C])
                    self.decay_av(3, qt, hg, G, (band_src, coarse_src),
                                  V, at, wts, oq)
                attn.append(at)
            return attn

    # ------------------------------------------------ out proj + LN
    def out_ln(self, lay, attn, res_tiles, hpool, spool, dram_out=None):
        nc, sb = self.nc, self.sb
        attnT = hpool.tile([128, 2, SEQ], BF16, tag=f"attnT{lay}")
        for st in range(NQT):
            nc.sync.dma_start_transpose(
                out=attnT[:, :, st * 128:(st + 1) * 128],
                in_=_opt(attn[st][:].rearrange("p h d -> p (h d)")))
        W = sb[f"WT_o{lay}"]
        out_tiles = []
        for st in range(NQT):
            ps = self.pst([128, D])
            for ih in range(2):
                nc.tensor.matmul(ps[:],
                                 _opt(attnT[:, ih, st * 128:(st + 1) * 128]),
                                 _opt(W[:, ih, :]), start=(ih == 0),
                                 stop=(ih == 1))
            res = res_tiles[st] if isinstance(res_tiles, list) else res_tiles
            x = spool.tile([128, D], F32, tag="lnx")
            nc.vector.tensor_tensor(out=x[:], in0=ps[:], in1=res[:], op=OP.add)
            stats = spool.tile([128, 6], F32, tag="bnst")
            mv = spool.tile([128, 2], F32, tag="bnmv")
            nc.vector.bn_stats(out=stats[:], in_=x[:])
            nc.vector.bn_aggr(out=mv[:], in_=stats[:])
            # 1/std = exp(-0.5*ln(var+eps)) -- stays in the ln/exp table set
            lv = spool.tile([128, 1], F32, tag="lv")
            nc.scalar.activation(out=lv[:], in_=_opt(mv[:, 1:2]), func=AF.Ln,
                                 bias=sb["eps"][:], scale=1.0)
            rstd = spool.tile([128, 1], F32, tag="rstd")
            nc.scalar.activation(out=rstd[:], in_=lv[:], func=AF.Exp,
                                 scale=-0.5)
            xn = spool.tile([128, D], F32, tag="lnxn")
            nc.vector.tensor_scalar(
                out=xn[:], in0=x[:], scalar1=_opt(mv[:, 0:1]), scalar2=rstd[:],
                op0=OP.subtract, op1=OP.mult)
            ho = hpool.tile([128, D], BF16, tag=f"h{lay}_{st}")
            nc.vector.tensor_tensor(out=ho[:], in0=xn[:],
                                    in1=sb[f"lng{lay}_r"][:], op=OP.mult)
            nc.vector.tensor_tensor(out=ho[:], in0=ho[:],
                                    in1=sb[f"lnb{lay}_r"][:], op=OP.add)
            if dram_out is not None:
                o32 = spool.tile([128, D], F32, tag="ho32", name="ho32")
                nc.vector.tensor_copy(out=o32[:], in_=ho[:])
                nc.sync.dma_start(out=dram_out[st * 128:(st + 1) * 128, :],
                                  in_=o32[:])
            out_tiles.append(ho)
        return out_tiles

    def transpose_bf(self, tiles, pool, tag):
        """bf16 natural tiles [8][128, D] -> [128, 2, SEQ] via DMA xbar."""
        nc = self.nc
        xT = pool.tile([128, 2, SEQ], BF16, tag=tag)
        for st in range(NQT):
            nc.sync.dma_start_transpose(
                out=xT[:, :, st * 128:(st + 1) * 128], in_=tiles[st][:])
        return xT

    # ------------------------------------------------ layer-3 prologue
    def l3_rows(self, h1T, mpool, lpool, dram_rows):
        nc, sb = self.nc, self.sb
        KT3 = self.proj_T(h1T, "WT_k3", "bk3_c", lpool, tag="KT3")
        c3 = mpool.tile([8, SEQ], F32, tag="c3")
        for scc in range(2):
            ps = self.pst([8, 512])
            for ih in range(2):
                nc.tensor.matmul(ps[:], _opt(sb["q3blk"][:, ih, :]),
                                 _opt(KT3[:, ih, scc * 512:(scc + 1) * 512]),
                                 start=(ih == 0), stop=(ih == 1))
            nc.vector.tensor_scalar_mul(
                out=_opt(c3[:, scc * 512:(scc + 1) * 512]), in0=ps[:],
                scalar1=ISQ)
        e3 = mpool.tile([8, SEQ], F32, tag="e3")
        nc.scalar.activation(out=e3[:], in_=c3[:], func=AF.Exp)
        cum3pad = lpool.tile([8, SEQ + 128], F32, tag="cum3pad")
        nc.vector.memset(_opt(cum3pad[:, 0:1]), 0.0)
        nc.vector.tensor_tensor_scan(
            out=_opt(cum3pad[:, 1:SEQ + 1]), data0=e3[:], data1=e3[:],
            initial=0.0, op0=OP.add, op1=OP.bypass)
        nc.vector.memset(_opt(cum3pad[:, SEQ + 1:]), 0.0)
        # rows -> DRAM -> partition-broadcast back
        c3b = mpool.tile([8, SEQ], F16, tag="c3b")
        nc.vector.tensor_copy(out=c3b[:], in_=c3[:])
        ncm = mpool.tile([8, SEQ], F32, tag="ncm")
        nc.vector.tensor_scalar_mul(out=ncm[:], in0=_opt(cum3pad[:, 1:SEQ + 1]),
                                    scalar1=-1.0)
        d_c3, d_nc = dram_rows
        nc.sync.dma_start(out=d_c3[:], in_=c3b[:])
        nc.sync.dma_start(out=d_nc[:], in_=ncm[:])
        c3bc = lpool.tile([128, H, SEQ], F16, tag="c3bc")
        dap = d_c3[:]
        nc.sync.dma_start(out=c3bc[:], in_=bass.AP(
            tensor=dap.tensor, offset=0,
            ap=[[0, 128], [SEQ, H], [1, SEQ]]))
        ncum = lpool.tile([128, H, SEQ], F32, tag="ncum")
        dap = d_nc[:]
        nc.sync.dma_start(out=ncum[:], in_=bass.AP(
            tensor=dap.tensor, offset=0,
            ap=[[0, 128], [SEQ, H], [1, SEQ]]))
        return c3bc, ncum, cum3pad

    # ------------------------------------------------ final mixture
    def final(self, hh, al_dram, out_dram, spool, tpool):
        nc, sb = self.nc, self.sb
        hhT = self.transpose_bf(hh, tpool, tag="hhT")
        for st in range(NQT):
            al = spool.tile([128, H], F32, tag="al", name="al")
            nc.sync.dma_start(out=al[:],
                              in_=al_dram[st * 128:(st + 1) * 128, :])
            acc = spool.tile([128, D], F32, tag="facc", name="facc")
            for h in range(H):
                vps = self.pst([128, D])
                o = (h % 4) * DK
                nc.tensor.matmul(
                    vps[:],
                    self.hslice(hhT, h, slice(st * 128, st * 128 + 128)),
                    _opt(sb["WlvT"][o:o + DK, :]),
                    start=True, stop=False,
                    tile_position=(o, 0),
                    skip_group_check=True)
                nc.tensor.matmul(
                    vps[:], _opt(sb["ONES4"][o:o + 1, :]),
                    _opt(sb["blv_r"][o:o + 1, :]),
                    start=False, stop=True, tile_position=(o, 0),
                    skip_group_check=True)
                vsb = spool.tile([128, D], BF16, tag="vsb", name="vsb")
                nc.scalar.activation(out=vsb[:], in_=vps[:], func=AF.Sigmoid)
                if h == 0:
                    nc.vector.tensor_scalar_mul(
                        out=acc[:], in0=vsb[:], scalar1=al[:, 0:1])
                else:
                    nc.vector.scalar_tensor_tensor(
                        out=acc[:], in0=vsb[:], scalar=al[:, h:h + 1],
                        in1=acc[:], op0=OP.mult, op1=OP.add)
            nc.sync.dma_start(out=out_dram[st * 128:(st + 1) * 128, :],
                              in_=acc[:])


class _StageDoneExc(Exception):
    pass


_StageDone = _StageDoneExc()


def _patched_act_tables(nc):
    import types
    from concourse.hw_specs import get_activation_tables
    import concourse.bass_primitives_rust as _bpr

    def patched(self):
        has_act = any(isinstance(i, mybir.InstActivation)
                      for b in self.main_func.blocks
                      for i in b.instructions)
        if not has_act:
            return
        tables = []
        for name, fns in get_activation_tables(self.m.arch).items():
            if name in ("exp_and_others", "natural_log", "exp_and_friends"):
                fns = set()
            tables.append((name, fns))
        from concourse import bacc as _bacc
        _bacc._bass_rust.insert_act_table_loads(self, tables)

    nc.insert_act_table_loads = types.MethodType(patched, nc)


def build(derived, debug=False, stage=None):
    stage = stage or os.environ.get("V2_STAGE", "full")
    nc = bacc.Bacc(None, target_bir_lowering=False)
    _patched_act_tables(nc)
    dd = {}
    for name, arr in derived.items():
        if name.startswith("_"):
            continue
        dt = {np.dtype(np.float32): F32, np.dtype(bf16): BF16,
              np.dtype(f16): F16}[np.dtype(arr.dtype)]
        dd[name] = nc.dram_tensor(name, list(arr.shape), dt,
                                  kind="ExternalInput")
    for nm, shape, dt in (("xbf_q", [SEQ, D], BF16), ("xbf_s", [SEQ, D], BF16),
                          ("xr_q", [SEQ, D], F32), ("xr_s", [SEQ, D], F32),
                          ("al_nat", [SEQ, H], F32)):
        dd[nm] = nc.dram_tensor(nm, shape, dt, kind="ExternalInput")
    out = nc.dram_tensor("out", [SEQ, D], F32, kind="ExternalOutput")
    d_c3 = nc.dram_tensor("rows_c3", [8, SEQ], F16, kind="Internal")
    d_nc = nc.dram_tensor("rows_nc", [8, SEQ], F32, kind="Internal")

    def dbg(name):
        return nc.dram_tensor(name, [SEQ, D], F32,
                              kind="ExternalOutput") if debug else None

    with tile.TileContext(nc) as tc, contextlib.ExitStack() as ctx:
      try:
        kb = KB(nc, tc, ctx)
        kb.pps = ctx.enter_context(
            tc.tile_pool(name="pps", bufs=2, space="PSUM"))
        kb.load_consts(dd)
        glob = ctx.enter_context(tc.tile_pool(name="glob", bufs=1))

        def dump_at(attn, name):
            if not debug:
                return
            t = nc.dram_tensor(name, [SEQ, D], F32, kind="ExternalOutput")
            with tc.tile_pool(name=f"dbg_{name}", bufs=2) as dp:
                for st in range(NQT):
                    o32 = dp.tile([128, D], F32, tag=f"d{st}")
                    nc.vector.tensor_copy(
                        out=o32[:].rearrange("p (h d) -> p h d", h=H),
                        in_=attn[st][:])
                    nc.sync.dma_start(out=t[st * 128:(st + 1) * 128, :],
                                      in_=o32[:])

        # ---------------- layer 1 (on x_q) ----------------
        with tc.tile_pool(name="r1", bufs=1) as r1, \
                tc.tile_pool(name="r1s", bufs=2) as r1s:
            xT_q = r1.tile([128, 2, SEQ], BF16, tag="xTq")
            nc.sync.dma_start_transpose(out=xT_q[:], in_=dd["xbf_q"][:])
            xr1 = []
            for st in range(NQT):
                t = r1.tile([128, D], F32, tag=f"xr1_{st}")
                nc.sync.dma_start(out=t[:],
                                  in_=dd["xr_q"][st * 128:(st + 1) * 128, :])
                xr1.append(t)
            QT1 = kb.proj_T(xT_q, "WT_q1", "bq1_c", r1, tag="QT1")
            V1 = kb.proj_V(xT_q, "WT_v1", "bv1_r", r1, tag="V1")
            attn1 = kb.attention(1, QT1, QT1, V1, r1)
            dump_at(attn1, "dbg_attn1")
            h1 = kb.out_ln(1, attn1, xr1, r1, r1s, dram_out=dbg("dbg_h1"))
            if stage == "l1":
                for st in range(NQT):
                    o32 = r1s.tile([128, D], F32, tag="s1o")
                    nc.vector.tensor_copy(out=o32[:], in_=h1[st][:])
                    nc.sync.dma_start(out=out[st * 128:(st + 1) * 128, :],
                                      in_=o32[:])
            h1T = kb.transpose_bf(h1, glob, tag="h1T")
        # ---------------- layer 2 (on x_s) ----------------
        with tc.tile_pool(name="r2", bufs=1) as r2, \
                tc.tile_pool(name="r2s", bufs=2) as r2s:
            if stage == "l1":
                raise _StageDone
            xT_s = r2.tile([128, 2, SEQ], BF16, tag="xTs")
            nc.sync.dma_start_transpose(out=xT_s[:], in_=dd["xbf_s"][:])
            xr2 = []
            for st in range(NQT):
                t = r2.tile([128, D], F32, tag=f"xr2_{st}")
                nc.sync.dma_start(out=t[:],
                                  in_=dd["xr_s"][st * 128:(st + 1) * 128, :])
                xr2.append(t)
            QT2 = kb.proj_T(xT_s, "WT_q2", "bq2_c", r2, tag="QT2")
            V2 = kb.proj_V(xT_s, "WT_v2", "bv2_r", r2, tag="V2")
            attn2 = kb.attention(2, QT2, QT2, V2, r2)
            h2 = kb.out_ln(2, attn2, xr2, r2, r2s, dram_out=dbg("dbg_h2"))
            if stage == "l2":
                for st in range(NQT):
                    o32 = r2s.tile([128, D], F32, tag="s2o")
                    nc.vector.tensor_copy(out=o32[:], in_=h2[st][:])
                    nc.sync.dma_start(out=out[st * 128:(st + 1) * 128, :],
                                      in_=o32[:])
            h2T = glob.tile([128, 2, SEQ], BF16, tag="h2T")
            for st in range(NQT):
                nc.sync.dma_start_transpose(
                    out=h2T[:, :, st * 128:(st + 1) * 128], in_=h2[st][:])
        # ---------------- layer 3 ----------------
        if stage == "l2":
            raise _StageDone
        lpool = ctx.enter_context(tc.tile_pool(name="l3pool", bufs=1))
        with tc.tile_pool(name="l3tmp", bufs=1) as l3tmp:
            V3 = kb.proj_V(h2T, "WT_v3", "bv3_r", lpool, tag="V3")
            c3bc, ncum, cum3pad = kb.l3_rows(h1T, l3tmp, lpool,
                                             (d_c3, d_nc))
            if stage == "pro":
                for st in range(NQT):
                    o32 = l3tmp.tile([128, D], F32, tag=f"s3o{st}")
                    nc.vector.tensor_copy(
                        out=o32[:].rearrange("p (h d) -> p h d", h=H),
                        in_=V3[st][:, :, 0:DK])
                    nc.sync.dma_start(out=out[st * 128:(st + 1) * 128, :],
                                      in_=o32[:])
        if stage == "pro":
            raise _StageDone
        with tc.tile_pool(name="r3", bufs=1) as r3, \
                tc.tile_pool(name="r3s", bufs=2) as r3s:
            attn3 = kb.attention3(V3, c3bc, ncum, cum3pad, r3)
            dump_at(attn3, "dbg_attn3")
            if stage == "a3":
                for st in range(NQT):
                    o32 = r3s.tile([128, D], F32, tag="s4o")
                    nc.vector.tensor_copy(
                        out=o32[:].rearrange("p (h d) -> p h d", h=H),
                        in_=attn3[st][:])
                    nc.sync.dma_start(out=out[st * 128:(st + 1) * 128, :],
                                      in_=o32[:])
                raise _StageDone
            hh = kb.out_ln(3, attn3, kb.sb["knowr_r"], r3, r3s,
                           dram_out=dbg("dbg_hh"))
            if stage == "hh":
                for st in range(NQT):
                    o32 = r3s.tile([128, D], F32, tag="s5o")
                    nc.vector.tensor_copy(out=o32[:], in_=hh[st][:])
                    nc.sync.dma_start(out=out[st * 128:(st + 1) * 128, :],
                                      in_=o32[:])
                raise _StageDone
            kb.final(hh, dd["al_nat"], out, r3s, r3)
      except _StageDoneExc:
        pass
    nc.compile()
    return nc


_CACHE = {}


def kernel(**inputs):
    drv = host_prep(inputs)
    if "nc" not in _CACHE:
        _CACHE["nc"] = build(drv)
    nc = _CACHE["nc"]
    in_maps = per_batch_maps(inputs, drv)
    from concourse.bass_utils import run_bass_kernel_spmd
    res = run_bass_kernel_spmd(nc, in_maps, core_ids=list(range(BS)))
    out = np.stack([np.asarray(res.results[b]["out"]) for b in range(BS)],
                   axis=0)
    return out.astype(np.float32)


if __name__ == "__main__":
    print("kernel module loaded OK")

